# revision 1
# baseline (speedup 1.0000x reference)
"""DetectionLoss Bass kernel for TRN2, 8-core SPMD.

Strategy:
- Device (identical program on all 8 cores; inputs differ only in the
  vocab slice of caption_logits):
  * build the (64,256) fused cost matrix (both samples stacked on the
    partition dim) from boxes + objectness,
  * run the 32-step greedy matching on the vector engine (per-row top-1
    via max/max_index, 32x32 stream transpose, per-sample argmax,
    dynamic-offset masking via registers),
  * per step, indirect-DMA-gather only the matched prediction's caption
    logit rows (30 rows of V/8 floats) - overlapping the big gather with
    the serial matching,
  * exp + free-dim accumulate on ACT -> per-(b,step,pos) partial sum(exp)
    over this core's vocab slice,
  * matched-pair L1/GIoU bbox loss and objectness BCE reduced to
    per-sample scalars on device.
- Host: shards caption_logits by vocab (plus small layout prep /
  broadcast of the box rows), all-reduces the per-core partial sumexps,
  takes log, gathers target-token logits, and combines the scalar
  losses (the final weighted mean).
"""

import sys

sys.path.insert(0, "/opt/trn_rl_repo")

import numpy as np

import concourse.bacc as bacc
import concourse.mybir as mybir
from concourse.bass import ds
from concourse.tile import TileContext

F32 = mybir.dt.float32
I32 = mybir.dt.int32
U32 = mybir.dt.uint32
Alu = mybir.AluOpType
Act = mybir.ActivationFunctionType

B, N, M, L = 2, 256, 32, 16
LM1 = L - 1  # 15 caption positions
S = M  # greedy steps
NEG = -1.0e9
EPS = 1e-7
ROWS_PER_STEP = B * LM1  # 30 gathered rows per step
STEPS_PER_BATCH = 4
NBATCH = S // STEPS_PER_BATCH  # 8 ACT sweeps over (120, V8)
GP = STEPS_PER_BATCH * ROWS_PER_STEP  # 120


def build_nc(V8: int, num_devices: int = 8):
    """Build the per-core Bass program. V8 = vocab slice width per core."""
    nc = bacc.Bacc(
        "TRN2", target_bir_lowering=False, debug=False, num_devices=num_devices
    )
    DVE = (mybir.EngineType.DVE,)
    DVESP = (mybir.EngineType.DVE, mybir.EngineType.SP)
    DVEACT = (mybir.EngineType.DVE, mybir.EngineType.SP)

    cl = nc.dram_tensor("cl", (B * N * L, V8), F32, kind="ExternalInput")
    # pbig: per (b,j) partition, 9 x 256 row segments:
    # [x1n y1n x2n y2n x1 y1 x2 y2 po]
    pbig = nc.dram_tensor("pbig", (64, 9 * N), F32, kind="ExternalInput")
    po = nc.dram_tensor("po", (B * N, 1), F32, kind="ExternalInput")
    pb = nc.dram_tensor("pb", (B * N, 4), F32, kind="ExternalInput")
    gb = nc.dram_tensor("gb", (B * M, 4), F32, kind="ExternalInput")
    out = nc.dram_tensor("out", (128, 16), F32, kind="ExternalOutput")

    # per-sample DRAM views for register-offset gathers
    cl2 = cl[:].rearrange("(b n l) v -> b n (l v)", b=B, n=N)  # (2, 256, L*V8)
    pbv = pb[:].rearrange("(b n) c -> b n c", b=B)
    gbv = gb[:].rearrange("(b m) c -> b m c", b=B)
    pov = po[:].rearrange("(b n) o -> b n o", b=B)

    with TileContext(nc) as tc:
        with (
            tc.tile_pool(name="cpool", bufs=1) as cp,
            tc.tile_pool(name="opool", bufs=4) as op,
            tc.tile_pool(name="gpool", bufs=3) as gp,
            tc.tile_pool(name="dpool", bufs=1) as dp,
        ):
            # ---------- input loads ----------
            pbig_sb = cp.tile([64, 9 * N], F32)
            nc.sync.dma_start(pbig_sb[:], pbig[:])

            def seg(k):
                return pbig_sb[:, k * N : (k + 1) * N]

            gb_sb = cp.tile([64, 4], F32)
            nc.sync.dma_start(gb_sb[:], gb[:])

            ts = nc.vector.tensor_scalar
            tt = nc.vector.tensor_tensor

            # ---------- cost matrix build ----------
            # gt cols (64,1)
            gx1n = cp.tile([64, 1], F32)
            gy1n = cp.tile([64, 1], F32)
            gx2n = cp.tile([64, 1], F32)
            gy2n = cp.tile([64, 1], F32)
            nc.vector.tensor_tensor(gx1n[:], gb_sb[:, 0:1], gb_sb[:, 2:3], op=Alu.min)
            nc.vector.tensor_tensor(gx2n[:], gb_sb[:, 0:1], gb_sb[:, 2:3], op=Alu.max)
            nc.vector.tensor_tensor(gy1n[:], gb_sb[:, 1:2], gb_sb[:, 3:4], op=Alu.min)
            nc.vector.tensor_tensor(gy2n[:], gb_sb[:, 1:2], gb_sb[:, 3:4], op=Alu.max)
            ga2 = cp.tile([64, 1], F32)
            gw = cp.tile([64, 1], F32)
            gh = cp.tile([64, 1], F32)
            nc.vector.tensor_tensor(gw[:], gx2n[:], gx1n[:], op=Alu.subtract)
            nc.vector.tensor_tensor(gh[:], gy2n[:], gy1n[:], op=Alu.subtract)
            nc.vector.tensor_tensor(ga2[:], gw[:], gh[:], op=Alu.mult)

            xi1 = cp.tile([64, N], F32)
            xi2 = cp.tile([64, N], F32)
            xe1 = cp.tile([64, N], F32)
            xe2 = cp.tile([64, N], F32)
            ts(xi1[:], seg(0), gx1n[:], None, op0=Alu.max)
            ts(xi2[:], seg(2), gx2n[:], None, op0=Alu.min)
            ts(xe1[:], seg(0), gx1n[:], None, op0=Alu.min)
            ts(xe2[:], seg(2), gx2n[:], None, op0=Alu.max)
            yi1 = cp.tile([64, N], F32)
            yi2 = cp.tile([64, N], F32)
            ye1 = cp.tile([64, N], F32)
            ye2 = cp.tile([64, N], F32)
            ts(yi1[:], seg(1), gy1n[:], None, op0=Alu.max)
            ts(yi2[:], seg(3), gy2n[:], None, op0=Alu.min)
            ts(ye1[:], seg(1), gy1n[:], None, op0=Alu.min)
            ts(ye2[:], seg(3), gy2n[:], None, op0=Alu.max)

            iw = cp.tile([64, N], F32)
            ih = cp.tile([64, N], F32)
            tt(iw[:], xi2[:], xi1[:], op=Alu.subtract)
            ts(iw[:], iw[:], 0.0, None, op0=Alu.max)
            tt(ih[:], yi2[:], yi1[:], op=Alu.subtract)
            ts(ih[:], ih[:], 0.0, None, op0=Alu.max)
            inter = cp.tile([64, N], F32)
            tt(inter[:], iw[:], ih[:], op=Alu.mult)

            ew = cp.tile([64, N], F32)
            eh = cp.tile([64, N], F32)
            tt(ew[:], xe2[:], xe1[:], op=Alu.subtract)
            tt(eh[:], ye2[:], ye1[:], op=Alu.subtract)
            enc = cp.tile([64, N], F32)
            tt(enc[:], ew[:], eh[:], op=Alu.mult)

            # a1 = (x2n-x1n)*(y2n-y1n); union = a1 + a2 - inter
            a1 = cp.tile([64, N], F32)
            a1h = cp.tile([64, N], F32)
            tt(a1[:], seg(2), seg(0), op=Alu.subtract)
            tt(a1h[:], seg(3), seg(1), op=Alu.subtract)
            tt(a1[:], a1[:], a1h[:], op=Alu.mult)
            union = cp.tile([64, N], F32)
            ts(union[:], a1[:], ga2[:], None, op0=Alu.add)
            tt(union[:], union[:], inter[:], op=Alu.subtract)

            iou = cp.tile([64, N], F32)
            tmp = cp.tile([64, N], F32)
            ts(tmp[:], union[:], EPS, None, op0=Alu.add)
            nc.vector.reciprocal(tmp[:], tmp[:])
            tt(iou[:], inter[:], tmp[:], op=Alu.mult)

            # giou = iou - (enc - union)/(enc + eps)
            giou = cp.tile([64, N], F32)
            tt(giou[:], enc[:], union[:], op=Alu.subtract)
            ts(tmp[:], enc[:], EPS, None, op0=Alu.add)
            nc.vector.reciprocal(tmp[:], tmp[:])
            tt(giou[:], giou[:], tmp[:], op=Alu.mult)
            tt(giou[:], iou[:], giou[:], op=Alu.subtract)

            # l1 from raw comps (segments 4..7)
            l1s = cp.tile([64, N], F32)
            dc = cp.tile([64, N], F32)
            for c in range(4):
                dst = l1s if c == 0 else dc
                ts(dst[:], seg(4 + c), gb_sb[:, c : c + 1], None,
                   op0=Alu.subtract)
                nc.scalar.activation(dst[:], dst[:], Act.Abs)
                if c > 0:
                    tt(l1s[:], l1s[:], dc[:], op=Alu.add)

            # objectness term: sigmoid(po) - 2 (po broadcast = segment 8)
            # sigmoid(x) = 1/(1+exp(-x)); only Exp/Ln/Identity share one
            # ACT table, so avoid Sigmoid/Softplus entirely.
            sig2 = cp.tile([64, N], F32)
            nc.scalar.activation(sig2[:], seg(8), Act.Exp, scale=-1.0)
            ts(sig2[:], sig2[:], 1.0, None, op0=Alu.add)
            nc.vector.reciprocal(sig2[:], sig2[:])
            ts(sig2[:], sig2[:], -2.0, None, op0=Alu.add)

            ncf = cp.tile([64, N], F32)
            tt(ncf[:], giou[:], l1s[:], op=Alu.subtract)
            tt(ncf[:], ncf[:], sig2[:], op=Alu.add)
            # split per-sample so dynamic-offset masking stays at partition 0
            ncA = cp.tile([64, N], F32)
            ncB = cp.tile([64, N], F32)
            nc.vector.tensor_copy(ncA[0:32, :], ncf[0:32, :])
            nc.vector.tensor_copy(ncB[0:32, :], ncf[32:64, :])

            # ---------- greedy matching ----------
            # Partition-layout rule: every compute AP must start at
            # partition 0/32/64/96. Sample b0 data lives at partition 0,
            # sample b1 at partition 32, throughout.
            # fully per-sample tile sets; every compute AP starts at
            # partition 0 (NCC_IBIR297: two-SB-input ops need equal bases)
            pkA = cp.tile([64, 32], F32)
            nc.vector.memset(pkA[:], 0.0)
            pkB = cp.tile([64, 32], F32)
            nc.vector.memset(pkB[:], 0.0)
            pk2A = cp.tile([64, 32], F32)
            nc.vector.memset(pk2A[:], 0.0)
            pk2B = cp.tile([64, 32], F32)
            nc.vector.memset(pk2B[:], 0.0)
            pkTA = cp.tile([64, 32], F32)
            pkTB = cp.tile([64, 32], F32)
            pk2TA = cp.tile([64, 32], F32)
            pk2TB = cp.tile([64, 32], F32)
            ridxA = cp.tile([64, 8], U32)
            ridxB = cp.tile([64, 8], U32)
            tmA = cp.tile([64, 32], F32)
            tmB = cp.tile([64, 32], F32)
            g8A = cp.tile([64, 8], F32)
            g8B = cp.tile([64, 8], F32)
            giA = cp.tile([64, 8], U32)
            giB = cp.tile([64, 8], U32)
            gtmA = cp.tile([64, 32], F32)
            nc.vector.memset(gtmA[:], 0.0)
            gtmB = cp.tile([64, 32], F32)
            nc.vector.memset(gtmB[:], 0.0)
            pisr = cp.tile([64, 32], F32)  # row0 = pis b0, row32 = pis b1
            nc.vector.memset(pisr[:], 0.0)
            gjsr = cp.tile([64, 32], F32)
            nc.vector.memset(gjsr[:], 0.0)
            pisri = cp.tile([64, 32], I32)  # write-once per-step int columns
            gjsri = cp.tile([64, 32], I32)

            outsb = cp.tile([128, 16], F32)
            nc.vector.memset(outsb[:], 0.0)
            mp = cp.tile([64, 4], F32)
            mg = cp.tile([64, 4], F32)
            pom = cp.tile([64, 1], F32)

            for s in range(S):
                nc.vector.max(pkA[0:32, 0:8], ncA[0:32, :])
                nc.vector.max_index(ridxA[0:32], pkA[0:32, 0:8], ncA[0:32, :])
                nc.vector.max(pkB[0:32, 0:8], ncB[0:32, :])
                nc.vector.max_index(ridxB[0:32], pkB[0:32, 0:8], ncB[0:32, :])
                nc.vector.tensor_copy(pk2A[0:32, 0:1], ridxA[0:32, 0:1])
                nc.vector.tensor_copy(pk2B[0:32, 0:1], ridxB[0:32, 0:1])
                nc.vector.transpose(pkTA[0:32, :], pkA[0:32, :])
                nc.vector.transpose(pkTB[0:32, :], pkB[0:32, :])
                nc.vector.transpose(pk2TA[0:32, :], pk2A[0:32, :])
                nc.vector.transpose(pk2TB[0:32, :], pk2B[0:32, :])
                tt(tmA[0:1], pkTA[0:1, 0:32], gtmA[0:1], op=Alu.add)
                tt(tmB[0:1], pkTB[0:1, 0:32], gtmB[0:1], op=Alu.add)
                nc.vector.max(g8A[0:1], tmA[0:1])
                nc.vector.max_index(giA[0:1], g8A[0:1], tmA[0:1])
                nc.vector.max(g8B[0:1], tmB[0:1])
                nc.vector.max_index(giB[0:1], g8B[0:1], tmB[0:1])
                nc.vector.tensor_copy(gjsri[0:1, s : s + 1], giA[0:1, 0:1])
                nc.vector.tensor_copy(gjsri[32:33, s : s + 1], giB[0:1, 0:1])
                j0 = nc.values_load(gjsri[0:1, s : s + 1], engines=DVESP,
                                    min_val=0, max_val=31,
                                    skip_runtime_bounds_check=True)
                j1 = nc.values_load(gjsri[32:33, s : s + 1], engines=DVESP,
                                    min_val=0, max_val=31,
                                    skip_runtime_bounds_check=True)
                i_f = op.tile([64, 1], F32, tag="i_f")
                nc.vector.tensor_copy(i_f[0:1], pk2TA[0:1, ds(j0, 1)])
                nc.vector.tensor_copy(i_f[32:33], pk2TB[0:1, ds(j1, 1)])
                nc.vector.tensor_copy(pisri[0:1, s : s + 1], i_f[0:1])
                nc.vector.tensor_copy(pisri[32:33, s : s + 1], i_f[32:33])
                i0 = nc.values_load(pisri[0:1, s : s + 1], engines=DVESP,
                                    min_val=0, max_val=N - 1,
                                    skip_runtime_bounds_check=True)
                i1 = nc.values_load(pisri[32:33, s : s + 1], engines=DVESP,
                                    min_val=0, max_val=N - 1,
                                    skip_runtime_bounds_check=True)
                nc.vector.memset(ncA[0:32, ds(i0, 1)], NEG)
                nc.vector.memset(ncB[0:32, ds(i1, 1)], NEG)
                nc.vector.memset(gtmA[0:1, ds(j0, 1)], NEG)
                nc.vector.memset(gtmB[0:1, ds(j1, 1)], NEG)
                nc.vector.tensor_copy(pisr[0:1, s : s + 1], i_f[0:1])
                nc.vector.tensor_copy(pisr[32:33, s : s + 1], i_f[32:33])
                nc.vector.tensor_copy(gjsr[0:1, s : s + 1], giA[0:1, 0:1])
                nc.vector.tensor_copy(gjsr[32:33, s : s + 1], giB[0:1, 0:1])

                # caption logit rows of the two matched preds: contiguous
                # (L-1)*V8 slabs fetched with register-offset DMAs (HWDGE).
                g, k = divmod(s, STEPS_PER_BATCH)
                if k == 0:
                    gtile = gp.tile([128, V8], F32, tag="gtile")
                base = k * ROWS_PER_STEP
                nc.sync.dma_start(
                    gtile[base : base + LM1, :],
                    cl2[0, ds(i0, 1), 0 : LM1 * V8])
                nc.sync.dma_start(
                    gtile[base + LM1 : base + ROWS_PER_STEP, :],
                    cl2[1, ds(i1, 1), 0 : LM1 * V8])
                # matched boxes / objectness, one row per step per sample
                nc.sync.dma_start(mp[s : s + 1, :], pbv[0, ds(i0, 1), :])
                nc.sync.dma_start(mp[32 + s : 33 + s, :], pbv[1, ds(i1, 1), :])
                nc.sync.dma_start(mg[s : s + 1, :], gbv[0, ds(j0, 1), :])
                nc.sync.dma_start(mg[32 + s : 33 + s, :], gbv[1, ds(j1, 1), :])
                nc.sync.dma_start(pom[s : s + 1, :], pov[0, ds(i0, 1), :])
                nc.sync.dma_start(pom[32 + s : 33 + s, :], pov[1, ds(i1, 1), :])
                if k == STEPS_PER_BATCH - 1:
                    dump = dp.tile([128, V8], F32, tag="dump")
                    nc.scalar.activation(dump[0:GP, :], gtile[0:GP, :], Act.Exp,
                                         accum_out=outsb[0:GP, g : g + 1])

            # ---------- post: pis/gjs columns via stream transpose ----------
            pgT = cp.tile([64, 32], F32)
            ggT = cp.tile([64, 32], F32)
            nc.vector.transpose(pgT[:], pisr[:])
            nc.vector.transpose(ggT[:], gjsr[:])
            # pgT[0:32,0] = pis b0; pgT[32:64,0] = pis b1
            nc.vector.tensor_copy(outsb[0:32, 8:9], pgT[0:32, 0:1])
            nc.vector.tensor_copy(outsb[32:64, 8:9], pgT[32:64, 0:1])
            nc.vector.tensor_copy(outsb[0:32, 9:10], ggT[0:32, 0:1])
            nc.vector.tensor_copy(outsb[32:64, 9:10], ggT[32:64, 0:1])

            # ---------- matched-pair bbox loss ----------
            md = cp.tile([64, 4], F32)
            l1p = cp.tile([64, 1], F32)
            tt(md[:], mp[:], mg[:], op=Alu.subtract)
            nc.scalar.activation(md[:], md[:], Act.Abs, accum_out=l1p[:])

            def col(t, c):
                return t[:, c : c + 1]

            mx1 = cp.tile([64, 1], F32)
            my1 = cp.tile([64, 1], F32)
            mx2 = cp.tile([64, 1], F32)
            my2 = cp.tile([64, 1], F32)
            tt(mx1[:], col(mp, 0), col(mp, 2), op=Alu.min)
            tt(mx2[:], col(mp, 0), col(mp, 2), op=Alu.max)
            tt(my1[:], col(mp, 1), col(mp, 3), op=Alu.min)
            tt(my2[:], col(mp, 1), col(mp, 3), op=Alu.max)
            nx1 = cp.tile([64, 1], F32)
            ny1 = cp.tile([64, 1], F32)
            nx2 = cp.tile([64, 1], F32)
            ny2 = cp.tile([64, 1], F32)
            tt(nx1[:], col(mg, 0), col(mg, 2), op=Alu.min)
            tt(nx2[:], col(mg, 0), col(mg, 2), op=Alu.max)
            tt(ny1[:], col(mg, 1), col(mg, 3), op=Alu.min)
            tt(ny2[:], col(mg, 1), col(mg, 3), op=Alu.max)

            w1 = cp.tile([64, 1], F32)
            w2 = cp.tile([64, 1], F32)
            w3 = cp.tile([64, 1], F32)
            w4 = cp.tile([64, 1], F32)
            tt(w1[:], mx1[:], nx1[:], op=Alu.max)  # xi1
            tt(w2[:], mx2[:], nx2[:], op=Alu.min)  # xi2
            tt(w2[:], w2[:], w1[:], op=Alu.subtract)
            ts(w2[:], w2[:], 0.0, None, op0=Alu.max)  # iw
            tt(w1[:], my1[:], ny1[:], op=Alu.max)
            tt(w3[:], my2[:], ny2[:], op=Alu.min)
            tt(w3[:], w3[:], w1[:], op=Alu.subtract)
            ts(w3[:], w3[:], 0.0, None, op0=Alu.max)  # ih
            minter = cp.tile([64, 1], F32)
            tt(minter[:], w2[:], w3[:], op=Alu.mult)
            tt(w1[:], mx2[:], mx1[:], op=Alu.subtract)
            tt(w2[:], my2[:], my1[:], op=Alu.subtract)
            tt(w1[:], w1[:], w2[:], op=Alu.mult)  # a1
            tt(w2[:], nx2[:], nx1[:], op=Alu.subtract)
            tt(w3[:], ny2[:], ny1[:], op=Alu.subtract)
            tt(w2[:], w2[:], w3[:], op=Alu.mult)  # a2
            munion = cp.tile([64, 1], F32)
            tt(munion[:], w1[:], w2[:], op=Alu.add)
            tt(munion[:], munion[:], minter[:], op=Alu.subtract)
            miou = cp.tile([64, 1], F32)
            ts(w1[:], munion[:], EPS, None, op0=Alu.add)
            nc.vector.reciprocal(w1[:], w1[:])
            tt(miou[:], minter[:], w1[:], op=Alu.mult)
            tt(w1[:], mx1[:], nx1[:], op=Alu.min)
            tt(w2[:], mx2[:], nx2[:], op=Alu.max)
            tt(w2[:], w2[:], w1[:], op=Alu.subtract)  # ew
            tt(w1[:], my1[:], ny1[:], op=Alu.min)
            tt(w3[:], my2[:], ny2[:], op=Alu.max)
            tt(w3[:], w3[:], w1[:], op=Alu.subtract)  # eh
            menc = cp.tile([64, 1], F32)
            tt(menc[:], w2[:], w3[:], op=Alu.mult)
            tt(w1[:], menc[:], munion[:], op=Alu.subtract)
            ts(w2[:], menc[:], EPS, None, op0=Alu.add)
            nc.vector.reciprocal(w2[:], w2[:])
            tt(w1[:], w1[:], w2[:], op=Alu.mult)
            mgiou = cp.tile([64, 1], F32)
            tt(mgiou[:], miou[:], w1[:], op=Alu.subtract)
            ts(w4[:], mgiou[:], -1.0, 1.0, op0=Alu.mult, op1=Alu.add)  # 1-giou

            # per-sample sums: transpose each (64,1) vector and accumulate
            # rows 0 / 32 separately.
            sums3 = cp.tile([64, 3], F32)  # col 0=l1, 1=1-g, 2=po; rows 0/32
            for ci, vec in enumerate((l1p[:], w4[:], pom[:])):
                pkx = cp.tile([64, 32], F32, tag="pkx")
                nc.vector.memset(pkx[:], 0.0)
                nc.vector.tensor_copy(pkx[:, 0:1], vec)
                pkxT = cp.tile([64, 32], F32, tag="pkxT")
                nc.vector.transpose(pkxT[:], pkx[:])
                ts(pkxT[0:1, :], pkxT[0:1, :], 0.0, None, op0=Alu.add,
                   op1=Alu.add, accum_out=sums3[0:1, ci : ci + 1])
                ts(pkxT[32:33, :], pkxT[32:33, :], 0.0, None, op0=Alu.add,
                   op1=Alu.add, accum_out=sums3[32:33, ci : ci + 1])

            # objectness base: relu(po) + ln(1+exp(-|po|)) on the broadcast
            # po slab (seg 8); rows 0 / 32 give the per-sample rowsums.
            relu = cp.tile([64, N], F32)
            abspo = cp.tile([64, N], F32)
            sp = cp.tile([64, N], F32)
            basesum = cp.tile([64, 1], F32)
            ts(relu[:], seg(8), 0.0, None, op0=Alu.max)
            nc.scalar.activation(abspo[:], seg(8), Act.Abs)
            nc.scalar.activation(sp[:], abspo[:], Act.Exp, scale=-1.0)
            ts(sp[:], sp[:], 1.0, None, op0=Alu.add)
            nc.scalar.activation(sp[:], sp[:], Act.Ln)
            tt(relu[:], relu[:], sp[:], op=Alu.add)
            ts(relu[:], relu[:], 0.0, None, op0=Alu.add, op1=Alu.add,
               accum_out=basesum[:])

            # bbox_b = clip(l1sum/128 + clip(gsum/32, 0, 2), 0)
            # obj_b = clip((basesum - pomsum)/256, 0)
            # per-sample results at rows 0 and 32 of outsb cols 10/11.
            b1t = cp.tile([64, 1], F32)
            b2t = cp.tile([64, 1], F32)
            obt = cp.tile([64, 1], F32)
            for b in range(2):
                r = 32 * b
                bb = slice(r, r + 1)
                ts(b1t[bb], sums3[bb, 0:1], 1.0 / 128.0, None, op0=Alu.mult)
                ts(b2t[bb], sums3[bb, 1:2], 1.0 / 32.0, None, op0=Alu.mult)
                ts(b2t[bb], b2t[bb], 0.0, 2.0, op0=Alu.max, op1=Alu.min)
                tt(b1t[bb], b1t[bb], b2t[bb], op=Alu.add)
                ts(b1t[bb], b1t[bb], 0.0, None, op0=Alu.max)
                tt(obt[bb], basesum[bb], sums3[bb, 2:3], op=Alu.subtract)
                ts(obt[bb], obt[bb], 1.0 / 256.0, 0.0, op0=Alu.mult, op1=Alu.max)
                nc.vector.tensor_copy(outsb[bb, 10:11], b1t[bb])
                nc.vector.tensor_copy(outsb[bb, 11:12], obt[bb])

            nc.sync.dma_start(out[:], outsb[:])

    nc.compile()
    return nc


# ---------------- host side ----------------

def shard_inputs(pred_boxes, pred_objectness, caption_logits, gt_boxes, V8, NC=8):
    pbf = pred_boxes.astype(np.float32)
    x1n = np.minimum(pbf[..., 0], pbf[..., 2])
    y1n = np.minimum(pbf[..., 1], pbf[..., 3])
    x2n = np.maximum(pbf[..., 0], pbf[..., 2])
    y2n = np.maximum(pbf[..., 1], pbf[..., 3])
    rows = np.stack(
        [x1n, y1n, x2n, y2n, pbf[..., 0], pbf[..., 1], pbf[..., 2], pbf[..., 3],
         pred_objectness.astype(np.float32)], axis=1)  # (B, 9, N)
    pbig = np.broadcast_to(rows[:, None, :, :], (B, M, 9, N)).reshape(64, 9 * N)
    pbig = np.ascontiguousarray(pbig)
    po = np.ascontiguousarray(pred_objectness.reshape(B * N, 1).astype(np.float32))
    pb = np.ascontiguousarray(pred_boxes.reshape(B * N, 4).astype(np.float32))
    gb = np.ascontiguousarray(gt_boxes.reshape(B * M, 4).astype(np.float32))
    clv = caption_logits.reshape(B * N * L, NC, V8)
    in_maps = []
    for c in range(NC):
        in_maps.append({
            "cl": np.ascontiguousarray(clv[:, c, :]).astype(np.float32, copy=False),
            "pbig": pbig, "po": po, "pb": pb, "gb": gb,
        })
    return in_maps


def combine(results, caption_logits, gt_tokens, V8, NC=8):
    """results: list of per-core 'out' arrays (128,16)."""
    out0 = results[0]
    sums = np.zeros((GP, NBATCH), np.float64)
    for c in range(NC):
        sums += results[c][0:GP, 0:NBATCH].astype(np.float64)
    lse = np.log(sums)  # (120, 8): row p = k*30 + b*15 + l, col g; step = 4g+k
    lse_bsl = (
        lse.reshape(STEPS_PER_BATCH, B, LM1, NBATCH)
        .transpose(1, 3, 0, 2)
        .reshape(B, S, LM1)
    )
    pis = out0[0:64, 8].astype(np.int64).reshape(2, 32)
    gjs = out0[0:64, 9].astype(np.int64).reshape(2, 32)
    tok = np.asarray(gt_tokens).astype(np.int64)

    bidx = np.arange(B)[:, None, None]
    lidx = np.arange(LM1)[None, None, :]
    tgt = tok[bidx, gjs[:, :, None], lidx + 1]  # (B, S, LM1)
    tlog = caption_logits[bidx, pis[:, :, None], lidx, tgt].astype(np.float64)
    ce = (lse_bsl - tlog).mean(axis=2)  # (B, S)
    cap = np.clip(np.clip(ce, 0.0, None).mean(axis=1), 0.0, None)  # (B,)
    bbox = out0[[0, 32], 10].astype(np.float64)
    obj = out0[[0, 32], 11].astype(np.float64)
    total = max((5.0 * bbox + 0.1 * cap + obj).mean(), 0.0)
    comps = [5.0 * bbox.mean(), 0.1 * cap.mean(), obj.mean()]
    return np.array([total] + comps, np.float32)


# ---------------- entry points ----------------

V8_FULL = 4000
NC_CORES = 8
_CACHE = {}


def get_nc(V8=V8_FULL):
    key = V8
    if key not in _CACHE:
        _CACHE[key] = build_nc(V8, num_devices=NC_CORES)
    return _CACHE[key]


def run_device(in_maps, V8=V8_FULL, trace=False, **kw):
    from concourse.bass_utils import run_bass_kernel_spmd

    nc = get_nc(V8)
    return run_bass_kernel_spmd(
        nc, in_maps, core_ids=list(range(NC_CORES)), trace=trace, **kw)


def kernel(pred_boxes, pred_objectness, caption_logits, gt_boxes, gt_tokens):
    pred_boxes = np.asarray(pred_boxes, np.float32)
    pred_objectness = np.asarray(pred_objectness, np.float32)
    caption_logits = np.asarray(caption_logits, np.float32)
    gt_boxes = np.asarray(gt_boxes, np.float32)
    in_maps = shard_inputs(
        pred_boxes, pred_objectness, caption_logits, gt_boxes, V8_FULL, NC_CORES)
    res = run_device(in_maps)
    outs = [r["out"] for r in res.results]
    return combine(outs, caption_logits, gt_tokens, V8_FULL, NC_CORES)



# revision 4
# speedup vs baseline: 1.2153x; 1.2153x over previous
"""DetectionLoss Bass kernel for TRN2, 8-core SPMD (v2).

Strategy (identical program on all 8 cores; inputs differ only in the
vocab slice of caption_logits):
- Build the (64,256) fused cost matrix (both samples stacked on the
  partition dim) from boxes + objectness.
- 32-step greedy matching entirely on DVE with zero registers and zero
  cross-engine hops on the critical chain. Key trick: broadcast the
  per-gt row max (and its argmax index) along the free dim BEFORE the
  32x32 stream transpose, so after the transpose every partition holds
  the full per-gt candidate row; the second-stage max and the one-hot
  index select then produce per-partition-broadcast results directly,
  which feed the iota-equality column mask of the cost matrix without
  any partition_broadcast or values_load.
- Per step, two register-offset HWDGE gathers (SP) fetch the matched
  predictions' caption-logit slabs (15 x V/8 floats each), overlapped
  with the serial matching; every 4 steps one ACT sweep computes
  exp + free-dim accumulate -> per-(b,step,pos) partial sum(exp).
- Matched boxes / objectness are recovered post-loop with one-hot
  tensor_tensor_reduce selects from SBUF (no DMAs), then the bbox
  L1/GIoU loss and objectness BCE reduce to per-sample scalars.
- Host: shards caption_logits by vocab, all-reduces the per-core
  partial sumexps, takes log, gathers target-token logits, and combines
  the scalar losses.
"""

import sys

sys.path.insert(0, "/opt/trn_rl_repo")

import numpy as np

import concourse.bacc as bacc
import concourse.mybir as mybir
from concourse.bass import ds
from concourse.tile import TileContext

F32 = mybir.dt.float32
I32 = mybir.dt.int32
U32 = mybir.dt.uint32
Alu = mybir.AluOpType
Act = mybir.ActivationFunctionType

B, N, M, L = 2, 256, 32, 16
LM1 = L - 1  # 15 caption positions
S = M  # greedy steps
NEG = -1.0e9
EPS = 1e-7
ROWS_PER_STEP = B * LM1  # 30 gathered rows per step
STEPS_PER_BATCH = 4
NBATCH = S // STEPS_PER_BATCH  # 8 ACT sweeps over (120, V8)
GP = STEPS_PER_BATCH * ROWS_PER_STEP  # 120


def build_nc(V8: int, num_devices: int = 8):
    """Build the per-core Bass program. V8 = vocab slice width per core."""
    nc = bacc.Bacc(
        "TRN2", target_bir_lowering=False, debug=False, num_devices=num_devices
    )
    SP = (mybir.EngineType.SP,)

    cl = nc.dram_tensor("cl", (B * N * L, V8), F32, kind="ExternalInput")
    # pbig: per (b,j) partition, 9 x 256 row segments:
    # [x1n y1n x2n y2n x1 y1 x2 y2 po]
    pbig = nc.dram_tensor("pbig", (64, 9 * N), F32, kind="ExternalInput")
    gb = nc.dram_tensor("gb", (B * M, 4), F32, kind="ExternalInput")
    # gbigT: raw gt coords broadcast along partitions, transposed layout:
    # [p, 32*c + j] = gt_boxes[p//32, j, c]
    gbigT = nc.dram_tensor("gbigT", (64, 4 * M), F32, kind="ExternalInput")
    # cst: host-built constants: cols 0-255 iota, col 256 partition idx mod 32
    cst = nc.dram_tensor("cst", (64, N + 1), F32, kind="ExternalInput")
    out = nc.dram_tensor("out", (128, 16), F32, kind="ExternalOutput")

    # per-sample DRAM view for register-offset caption gathers
    cl2 = cl[:].rearrange("(b n l) v -> b n (l v)", b=B, n=N)  # (2, 256, L*V8)

    with TileContext(nc) as tc:
        with (
            tc.tile_pool(name="cpool", bufs=1) as cp,
            tc.tile_pool(name="gpool", bufs=3) as gp,
            tc.tile_pool(name="dpool", bufs=1) as dp,
        ):
            # ---------- input loads ----------
            pbig_sb = cp.tile([64, 9 * N], F32)
            nc.sync.dma_start(pbig_sb[:], pbig[:])

            def seg(k):
                return pbig_sb[:, k * N : (k + 1) * N]

            gb_sb = cp.tile([64, 4], F32)
            nc.sync.dma_start(gb_sb[:], gb[:])
            gbigT_sb = cp.tile([64, 4 * M], F32)
            nc.sync.dma_start(gbigT_sb[:], gbigT[:])

            ts = nc.vector.tensor_scalar
            tt = nc.vector.tensor_tensor
            ttr = nc.vector.tensor_tensor_reduce

            # ---------- constant tiles (host-supplied iotas) ----------
            cst_sb = cp.tile([64, N + 1], F32)
            nc.sync.dma_start(cst_sb[:], cst[:])
            iota256f = cst_sb[:, 0:N]
            iota32f = cst_sb[:, 0:32]
            iotaPf = cst_sb[:, N : N + 1]
            z32 = cp.tile([64, 32], F32)
            nc.vector.memset(z32[:], 0.0)
            negrow = cp.tile([64, N], F32)
            nc.vector.memset(negrow[:], NEG)

            # ---------- cost matrix build ----------
            # gt cols (64,1)
            gx1n = cp.tile([64, 1], F32)
            gy1n = cp.tile([64, 1], F32)
            gx2n = cp.tile([64, 1], F32)
            gy2n = cp.tile([64, 1], F32)
            tt(gx1n[:], gb_sb[:, 0:1], gb_sb[:, 2:3], op=Alu.min)
            tt(gx2n[:], gb_sb[:, 0:1], gb_sb[:, 2:3], op=Alu.max)
            tt(gy1n[:], gb_sb[:, 1:2], gb_sb[:, 3:4], op=Alu.min)
            tt(gy2n[:], gb_sb[:, 1:2], gb_sb[:, 3:4], op=Alu.max)
            ga2 = cp.tile([64, 1], F32)
            gw = cp.tile([64, 1], F32)
            gh = cp.tile([64, 1], F32)
            tt(gw[:], gx2n[:], gx1n[:], op=Alu.subtract)
            tt(gh[:], gy2n[:], gy1n[:], op=Alu.subtract)
            tt(ga2[:], gw[:], gh[:], op=Alu.mult)

            xi1 = cp.tile([64, N], F32)
            xi2 = cp.tile([64, N], F32)
            xe1 = cp.tile([64, N], F32)
            xe2 = cp.tile([64, N], F32)
            ts(xi1[:], seg(0), gx1n[:], None, op0=Alu.max)
            ts(xi2[:], seg(2), gx2n[:], None, op0=Alu.min)
            ts(xe1[:], seg(0), gx1n[:], None, op0=Alu.min)
            ts(xe2[:], seg(2), gx2n[:], None, op0=Alu.max)
            yi1 = cp.tile([64, N], F32)
            yi2 = cp.tile([64, N], F32)
            ye1 = cp.tile([64, N], F32)
            ye2 = cp.tile([64, N], F32)
            ts(yi1[:], seg(1), gy1n[:], None, op0=Alu.max)
            ts(yi2[:], seg(3), gy2n[:], None, op0=Alu.min)
            ts(ye1[:], seg(1), gy1n[:], None, op0=Alu.min)
            ts(ye2[:], seg(3), gy2n[:], None, op0=Alu.max)

            iw = cp.tile([64, N], F32)
            ih = cp.tile([64, N], F32)
            tt(iw[:], xi2[:], xi1[:], op=Alu.subtract)
            ts(iw[:], iw[:], 0.0, None, op0=Alu.max)
            tt(ih[:], yi2[:], yi1[:], op=Alu.subtract)
            ts(ih[:], ih[:], 0.0, None, op0=Alu.max)
            inter = cp.tile([64, N], F32)
            tt(inter[:], iw[:], ih[:], op=Alu.mult)

            ew = cp.tile([64, N], F32)
            eh = cp.tile([64, N], F32)
            tt(ew[:], xe2[:], xe1[:], op=Alu.subtract)
            tt(eh[:], ye2[:], ye1[:], op=Alu.subtract)
            enc = cp.tile([64, N], F32)
            tt(enc[:], ew[:], eh[:], op=Alu.mult)

            # a1 = (x2n-x1n)*(y2n-y1n); union = a1 + a2 - inter
            a1 = cp.tile([64, N], F32)
            a1h = cp.tile([64, N], F32)
            tt(a1[:], seg(2), seg(0), op=Alu.subtract)
            tt(a1h[:], seg(3), seg(1), op=Alu.subtract)
            tt(a1[:], a1[:], a1h[:], op=Alu.mult)
            union = cp.tile([64, N], F32)
            ts(union[:], a1[:], ga2[:], None, op0=Alu.add)
            tt(union[:], union[:], inter[:], op=Alu.subtract)

            iou = cp.tile([64, N], F32)
            tmp = cp.tile([64, N], F32)
            ts(tmp[:], union[:], EPS, None, op0=Alu.add)
            nc.vector.reciprocal(tmp[:], tmp[:])
            tt(iou[:], inter[:], tmp[:], op=Alu.mult)

            # giou = iou - (enc - union)/(enc + eps)
            giou = cp.tile([64, N], F32)
            tt(giou[:], enc[:], union[:], op=Alu.subtract)
            ts(tmp[:], enc[:], EPS, None, op0=Alu.add)
            nc.vector.reciprocal(tmp[:], tmp[:])
            tt(giou[:], giou[:], tmp[:], op=Alu.mult)
            tt(giou[:], iou[:], giou[:], op=Alu.subtract)

            # l1 from raw comps (segments 4..7)
            l1s = cp.tile([64, N], F32)
            dc = cp.tile([64, N], F32)
            for c in range(4):
                dst = l1s if c == 0 else dc
                ts(dst[:], seg(4 + c), gb_sb[:, c : c + 1], None,
                   op0=Alu.subtract)
                nc.scalar.activation(dst[:], dst[:], Act.Abs)
                if c > 0:
                    tt(l1s[:], l1s[:], dc[:], op=Alu.add)

            # objectness term: sigmoid(po) - 2 (po broadcast = segment 8)
            sig2 = cp.tile([64, N], F32)
            nc.scalar.activation(sig2[:], seg(8), Act.Exp, scale=-1.0)
            ts(sig2[:], sig2[:], 1.0, None, op0=Alu.add)
            nc.vector.reciprocal(sig2[:], sig2[:])
            ts(sig2[:], sig2[:], -2.0, None, op0=Alu.add)

            # ncf = giou - l1 + (sigmoid - 2)  (value to MAXIMIZE)
            ncf = cp.tile([64, N], F32)
            tt(ncf[:], giou[:], l1s[:], op=Alu.subtract)
            tt(ncf[:], ncf[:], sig2[:], op=Alu.add)

            # ---------- greedy matching state ----------
            pk = cp.tile([64, 32], F32)      # top-8 per gt row (cols 0-7)
            ridx = cp.tile([64, 32], U32)    # argmax indices (cols 0-7)
            ridxf = cp.tile([64, 1], F32)
            pk0m = cp.tile([64, 1], F32)
            vstag = cp.tile([64, 32], F32)
            istag = cp.tile([64, 32], F32)
            vstagT = cp.tile([64, 32], F32)
            istagT = cp.tile([64, 32], F32)
            g8 = cp.tile([64, 8], F32)
            gi = cp.tile([64, 8], U32)
            jf = cp.tile([64, 1], F32)
            ohj = cp.tile([64, 32], F32)
            dump32 = cp.tile([64, 32], F32)
            if_ = cp.tile([64, 1], F32)
            pen = cp.tile([64, N], F32)
            ohp = cp.tile([64, 1], F32)
            gmaskP = cp.tile([64, 1], F32)
            nc.vector.memset(gmaskP[:], 0.0)
            pisr = cp.tile([64, 32], F32)
            gjsr = cp.tile([64, 32], F32)
            pisri32 = cp.tile([64, 32], I32)

            outsb = cp.tile([128, 16], F32)
            nc.vector.memset(outsb[:], 0.0)

            # ---------- greedy matching loop ----------
            for s in range(S):
                nc.vector.max(pk[:, 0:8], ncf[:])
                nc.vector.max_index(ridx[:, 0:8], pk[:, 0:8], ncf[:])
                nc.vector.tensor_copy(ridxf[:], ridx[:, 0:1])
                # fold gt-row mask into the stage-2 candidates
                tt(pk0m[:], pk[:, 0:1], gmaskP[:], op=Alu.add)
                # broadcast along free dim so the transpose fills every row
                ts(vstag[:], z32[:], pk0m[:], None, op0=Alu.add)
                ts(istag[:], z32[:], ridxf[:], None, op0=Alu.add)
                nc.vector.transpose(vstagT[:], vstag[:])
                nc.vector.transpose(istagT[:], istag[:])
                # stage 2: winner gt (j) per sample, on every partition
                nc.vector.max(g8[:], vstagT[:])
                nc.vector.max_index(gi[:], g8[:], vstagT[:])
                nc.vector.tensor_copy(jf[:], gi[:, 0:1])
                # one-hot of j along free dim; select i = ridx[j]
                ts(ohj[:], iota32f, jf[:], None, op0=Alu.is_equal)
                tt(dump32[:], istagT[:], ohj[:], op=Alu.mult)
                ts(dump32[:], dump32[:], 0.0, None, op0=Alu.add,
                   op1=Alu.add, accum_out=if_[:])
                # mask pred column i in ncf (per-partition bcast via if_)
                ts(pen[:], iota256f, if_[:], None, op0=Alu.is_equal)
                tt(pen[:], pen[:], negrow[:], op=Alu.mult)
                tt(ncf[:], ncf[:], pen[:], op=Alu.add)
                # mask gt j for stage-2 of later steps
                ts(ohp[:], iotaPf, jf[:], None, op0=Alu.is_equal)
                tt(ohp[:], ohp[:], negrow[:, 0:1], op=Alu.mult)
                tt(gmaskP[:], gmaskP[:], ohp[:], op=Alu.add)
                # record
                nc.vector.tensor_copy(pisr[:, s : s + 1], if_[:])
                nc.vector.tensor_copy(gjsr[:, s : s + 1], jf[:])
                nc.vector.tensor_copy(pisri32[:, s : s + 1], if_[:])

                i0 = nc.values_load(pisri32[0:1, s : s + 1], engines=SP,
                                    min_val=0, max_val=N - 1,
                                    skip_runtime_bounds_check=True)
                i1 = nc.values_load(pisri32[32:33, s : s + 1], engines=SP,
                                    min_val=0, max_val=N - 1,
                                    skip_runtime_bounds_check=True)
                # caption logit rows of the two matched preds: contiguous
                # (L-1)*V8 slabs fetched with register-offset DMAs (HWDGE).
                g, k = divmod(s, STEPS_PER_BATCH)
                if k == 0:
                    gtile = gp.tile([128, V8], F32, tag="gtile")
                base = k * ROWS_PER_STEP
                nc.sync.dma_start(
                    gtile[base : base + LM1, :],
                    cl2[0, ds(i0, 1), 0 : LM1 * V8])
                nc.sync.dma_start(
                    gtile[base + LM1 : base + ROWS_PER_STEP, :],
                    cl2[1, ds(i1, 1), 0 : LM1 * V8])
                if k == STEPS_PER_BATCH - 1:
                    dump = dp.tile([128, V8], F32, tag="dump")
                    nc.scalar.activation(dump[0:GP, :], gtile[0:GP, :], Act.Exp,
                                         accum_out=outsb[0:GP, g : g + 1])

            # ---------- post: pis/gjs columns via stream transpose ----------
            pgT = cp.tile([64, 32], F32)
            ggT = cp.tile([64, 32], F32)
            nc.vector.transpose(pgT[:], pisr[:])
            nc.vector.transpose(ggT[:], gjsr[:])
            # pgT[0:32,0] = pis b0; pgT[32:64,0] = pis b1
            nc.vector.tensor_copy(outsb[0:32, 8:9], pgT[0:32, 0:1])
            nc.vector.tensor_copy(outsb[32:64, 8:9], pgT[32:64, 0:1])
            nc.vector.tensor_copy(outsb[0:32, 9:10], ggT[0:32, 0:1])
            nc.vector.tensor_copy(outsb[32:64, 9:10], ggT[32:64, 0:1])

            # ---------- matched boxes via one-hot selects (no DMA) ----------
            # rows 0-31 = sample A steps, 32-63 = sample B steps
            mp = cp.tile([64, 4], F32)
            mg = cp.tile([64, 4], F32)
            pom = cp.tile([64, 1], F32)
            ohA = cp.tile([64, N], F32)
            dump256 = cp.tile([64, N], F32)
            ts(ohA[:], iota256f, pgT[:, 0:1], None, op0=Alu.is_equal)
            for c in range(4):
                tt(dump256[:], ohA[:], seg(4 + c), op=Alu.mult)
                ts(dump256[:], dump256[:], 0.0, None, op0=Alu.add,
                   op1=Alu.add, accum_out=mp[:, c : c + 1])
            tt(dump256[:], ohA[:], seg(8), op=Alu.mult)
            ts(dump256[:], dump256[:], 0.0, None, op0=Alu.add,
               op1=Alu.add, accum_out=pom[:])
            ohG = cp.tile([64, 32], F32)
            ts(ohG[:], iota32f, ggT[:, 0:1], None, op0=Alu.is_equal)
            for c in range(4):
                tt(dump32[:], ohG[:], gbigT_sb[:, c * M : (c + 1) * M],
                   op=Alu.mult)
                ts(dump32[:], dump32[:], 0.0, None, op0=Alu.add,
                   op1=Alu.add, accum_out=mg[:, c : c + 1])

            # ---------- matched-pair bbox loss ----------
            md = cp.tile([64, 4], F32)
            l1p = cp.tile([64, 1], F32)
            tt(md[:], mp[:], mg[:], op=Alu.subtract)
            nc.scalar.activation(md[:], md[:], Act.Abs, accum_out=l1p[:])

            def col(t, c):
                return t[:, c : c + 1]

            mx1 = cp.tile([64, 1], F32)
            my1 = cp.tile([64, 1], F32)
            mx2 = cp.tile([64, 1], F32)
            my2 = cp.tile([64, 1], F32)
            tt(mx1[:], col(mp, 0), col(mp, 2), op=Alu.min)
            tt(mx2[:], col(mp, 0), col(mp, 2), op=Alu.max)
            tt(my1[:], col(mp, 1), col(mp, 3), op=Alu.min)
            tt(my2[:], col(mp, 1), col(mp, 3), op=Alu.max)
            nx1 = cp.tile([64, 1], F32)
            ny1 = cp.tile([64, 1], F32)
            nx2 = cp.tile([64, 1], F32)
            ny2 = cp.tile([64, 1], F32)
            tt(nx1[:], col(mg, 0), col(mg, 2), op=Alu.min)
            tt(nx2[:], col(mg, 0), col(mg, 2), op=Alu.max)
            tt(ny1[:], col(mg, 1), col(mg, 3), op=Alu.min)
            tt(ny2[:], col(mg, 1), col(mg, 3), op=Alu.max)

            w1 = cp.tile([64, 1], F32)
            w2 = cp.tile([64, 1], F32)
            w3 = cp.tile([64, 1], F32)
            w4 = cp.tile([64, 1], F32)
            tt(w1[:], mx1[:], nx1[:], op=Alu.max)  # xi1
            tt(w2[:], mx2[:], nx2[:], op=Alu.min)  # xi2
            tt(w2[:], w2[:], w1[:], op=Alu.subtract)
            ts(w2[:], w2[:], 0.0, None, op0=Alu.max)  # iw
            tt(w1[:], my1[:], ny1[:], op=Alu.max)
            tt(w3[:], my2[:], ny2[:], op=Alu.min)
            tt(w3[:], w3[:], w1[:], op=Alu.subtract)
            ts(w3[:], w3[:], 0.0, None, op0=Alu.max)  # ih
            minter = cp.tile([64, 1], F32)
            tt(minter[:], w2[:], w3[:], op=Alu.mult)
            tt(w1[:], mx2[:], mx1[:], op=Alu.subtract)
            tt(w2[:], my2[:], my1[:], op=Alu.subtract)
            tt(w1[:], w1[:], w2[:], op=Alu.mult)  # a1
            tt(w2[:], nx2[:], nx1[:], op=Alu.subtract)
            tt(w3[:], ny2[:], ny1[:], op=Alu.subtract)
            tt(w2[:], w2[:], w3[:], op=Alu.mult)  # a2
            munion = cp.tile([64, 1], F32)
            tt(munion[:], w1[:], w2[:], op=Alu.add)
            tt(munion[:], munion[:], minter[:], op=Alu.subtract)
            miou = cp.tile([64, 1], F32)
            ts(w1[:], munion[:], EPS, None, op0=Alu.add)
            nc.vector.reciprocal(w1[:], w1[:])
            tt(miou[:], minter[:], w1[:], op=Alu.mult)
            tt(w1[:], mx1[:], nx1[:], op=Alu.min)
            tt(w2[:], mx2[:], nx2[:], op=Alu.max)
            tt(w2[:], w2[:], w1[:], op=Alu.subtract)  # ew
            tt(w1[:], my1[:], ny1[:], op=Alu.min)
            tt(w3[:], my2[:], ny2[:], op=Alu.max)
            tt(w3[:], w3[:], w1[:], op=Alu.subtract)  # eh
            menc = cp.tile([64, 1], F32)
            tt(menc[:], w2[:], w3[:], op=Alu.mult)
            tt(w1[:], menc[:], munion[:], op=Alu.subtract)
            ts(w2[:], menc[:], EPS, None, op0=Alu.add)
            nc.vector.reciprocal(w2[:], w2[:])
            tt(w1[:], w1[:], w2[:], op=Alu.mult)
            mgiou = cp.tile([64, 1], F32)
            tt(mgiou[:], miou[:], w1[:], op=Alu.subtract)
            ts(w4[:], mgiou[:], -1.0, 1.0, op0=Alu.mult, op1=Alu.add)  # 1-giou

            # per-sample sums: transpose each (64,1) vector and accumulate
            # rows 0 / 32 separately.
            sums3 = cp.tile([64, 3], F32)  # col 0=l1, 1=1-g, 2=po; rows 0/32
            for ci, vec in enumerate((l1p[:], w4[:], pom[:])):
                pkx = cp.tile([64, 32], F32, tag="pkx")
                nc.vector.memset(pkx[:], 0.0)
                nc.vector.tensor_copy(pkx[:, 0:1], vec)
                pkxT = cp.tile([64, 32], F32, tag="pkxT")
                nc.vector.transpose(pkxT[:], pkx[:])
                ts(pkxT[0:1, :], pkxT[0:1, :], 0.0, None, op0=Alu.add,
                   op1=Alu.add, accum_out=sums3[0:1, ci : ci + 1])
                ts(pkxT[32:33, :], pkxT[32:33, :], 0.0, None, op0=Alu.add,
                   op1=Alu.add, accum_out=sums3[32:33, ci : ci + 1])

            # objectness base: relu(po) + ln(1+exp(-|po|)) on the broadcast
            # po slab (seg 8); rows 0 / 32 give the per-sample rowsums.
            relu = cp.tile([64, N], F32)
            abspo = cp.tile([64, N], F32)
            sp = cp.tile([64, N], F32)
            basesum = cp.tile([64, 1], F32)
            ts(relu[:], seg(8), 0.0, None, op0=Alu.max)
            nc.scalar.activation(abspo[:], seg(8), Act.Abs)
            nc.scalar.activation(sp[:], abspo[:], Act.Exp, scale=-1.0)
            ts(sp[:], sp[:], 1.0, None, op0=Alu.add)
            nc.scalar.activation(sp[:], sp[:], Act.Ln)
            tt(relu[:], relu[:], sp[:], op=Alu.add)
            ts(relu[:], relu[:], 0.0, None, op0=Alu.add, op1=Alu.add,
               accum_out=basesum[:])

            # bbox_b = clip(l1sum/128 + clip(gsum/32, 0, 2), 0)
            # obj_b = clip((basesum - pomsum)/256, 0)
            # per-sample results at rows 0 and 32 of outsb cols 10/11.
            b1t = cp.tile([64, 1], F32)
            b2t = cp.tile([64, 1], F32)
            obt = cp.tile([64, 1], F32)
            for b in range(2):
                r = 32 * b
                bb = slice(r, r + 1)
                ts(b1t[bb], sums3[bb, 0:1], 1.0 / 128.0, None, op0=Alu.mult)
                ts(b2t[bb], sums3[bb, 1:2], 1.0 / 32.0, None, op0=Alu.mult)
                ts(b2t[bb], b2t[bb], 0.0, 2.0, op0=Alu.max, op1=Alu.min)
                tt(b1t[bb], b1t[bb], b2t[bb], op=Alu.add)
                ts(b1t[bb], b1t[bb], 0.0, None, op0=Alu.max)
                tt(obt[bb], basesum[bb], sums3[bb, 2:3], op=Alu.subtract)
                ts(obt[bb], obt[bb], 1.0 / 256.0, 0.0, op0=Alu.mult, op1=Alu.max)
                nc.vector.tensor_copy(outsb[bb, 10:11], b1t[bb])
                nc.vector.tensor_copy(outsb[bb, 11:12], obt[bb])

            nc.sync.dma_start(out[:], outsb[:])

    nc.compile()
    return nc


# ---------------- host side ----------------

def shard_inputs(pred_boxes, pred_objectness, caption_logits, gt_boxes, V8, NC=8):
    pbf = pred_boxes.astype(np.float32)
    x1n = np.minimum(pbf[..., 0], pbf[..., 2])
    y1n = np.minimum(pbf[..., 1], pbf[..., 3])
    x2n = np.maximum(pbf[..., 0], pbf[..., 2])
    y2n = np.maximum(pbf[..., 1], pbf[..., 3])
    rows = np.stack(
        [x1n, y1n, x2n, y2n, pbf[..., 0], pbf[..., 1], pbf[..., 2], pbf[..., 3],
         pred_objectness.astype(np.float32)], axis=1)  # (B, 9, N)
    pbig = np.broadcast_to(rows[:, None, :, :], (B, M, 9, N)).reshape(64, 9 * N)
    pbig = np.ascontiguousarray(pbig)
    gb = np.ascontiguousarray(gt_boxes.reshape(B * M, 4).astype(np.float32))
    gbf = gt_boxes.astype(np.float32)  # (B, M, 4)
    gbigT = np.zeros((64, 4 * M), np.float32)
    for b in range(B):
        for c in range(4):
            gbigT[32 * b : 32 * b + 32, c * M : (c + 1) * M] = gbf[b, :, c][None, :]
    cstv = np.zeros((64, N + 1), np.float32)
    cstv[:, 0:N] = np.arange(N, dtype=np.float32)[None, :]
    cstv[:, N] = (np.arange(64) % 32).astype(np.float32)
    clv = caption_logits.reshape(B * N * L, NC, V8)
    in_maps = []
    for c in range(NC):
        in_maps.append({
            "cl": np.ascontiguousarray(clv[:, c, :]).astype(np.float32, copy=False),
            "pbig": pbig, "gb": gb, "gbigT": gbigT, "cst": cstv,
        })
    return in_maps


def combine(results, caption_logits, gt_tokens, V8, NC=8):
    """results: list of per-core 'out' arrays (128,16)."""
    out0 = results[0]
    sums = np.zeros((GP, NBATCH), np.float64)
    for c in range(NC):
        sums += results[c][0:GP, 0:NBATCH].astype(np.float64)
    lse = np.log(sums)  # (120, 8): row p = k*30 + b*15 + l, col g; step = 4g+k
    lse_bsl = (
        lse.reshape(STEPS_PER_BATCH, B, LM1, NBATCH)
        .transpose(1, 3, 0, 2)
        .reshape(B, S, LM1)
    )
    pis = out0[0:64, 8].astype(np.int64).reshape(2, 32)
    gjs = out0[0:64, 9].astype(np.int64).reshape(2, 32)
    tok = np.asarray(gt_tokens).astype(np.int64)

    bidx = np.arange(B)[:, None, None]
    lidx = np.arange(LM1)[None, None, :]
    tgt = tok[bidx, gjs[:, :, None], lidx + 1]  # (B, S, LM1)
    tlog = caption_logits[bidx, pis[:, :, None], lidx, tgt].astype(np.float64)
    ce = (lse_bsl - tlog).mean(axis=2)  # (B, S)
    cap = np.clip(np.clip(ce, 0.0, None).mean(axis=1), 0.0, None)  # (B,)
    bbox = out0[[0, 32], 10].astype(np.float64)
    obj = out0[[0, 32], 11].astype(np.float64)
    total = max((5.0 * bbox + 0.1 * cap + obj).mean(), 0.0)
    comps = [5.0 * bbox.mean(), 0.1 * cap.mean(), obj.mean()]
    return np.array([total] + comps, np.float32)


# ---------------- entry points ----------------

V8_FULL = 4000
NC_CORES = 8
_CACHE = {}


def get_nc(V8=V8_FULL):
    key = V8
    if key not in _CACHE:
        _CACHE[key] = build_nc(V8, num_devices=NC_CORES)
    return _CACHE[key]


def run_device(in_maps, V8=V8_FULL, trace=False, **kw):
    from concourse.bass_utils import run_bass_kernel_spmd

    nc = get_nc(V8)
    return run_bass_kernel_spmd(
        nc, in_maps, core_ids=list(range(NC_CORES)), trace=trace, **kw)


def kernel(pred_boxes, pred_objectness, caption_logits, gt_boxes, gt_tokens):
    pred_boxes = np.asarray(pred_boxes, np.float32)
    pred_objectness = np.asarray(pred_objectness, np.float32)
    caption_logits = np.asarray(caption_logits, np.float32)
    gt_boxes = np.asarray(gt_boxes, np.float32)
    in_maps = shard_inputs(
        pred_boxes, pred_objectness, caption_logits, gt_boxes, V8_FULL, NC_CORES)
    res = run_device(in_maps)
    outs = [r["out"] for r in res.results]
    return combine(outs, caption_logits, gt_tokens, V8_FULL, NC_CORES)


# revision 7
# speedup vs baseline: 1.9238x; 1.5830x over previous
"""DetectionLoss Bass kernel for TRN2, 8-core SPMD (v2).

Strategy (identical program on all 8 cores; inputs differ only in the
vocab slice of caption_logits):
- Build the (64,256) fused cost matrix (both samples stacked on the
  partition dim) from boxes + objectness.
- 32-step greedy matching entirely on DVE with zero registers and zero
  cross-engine hops on the critical chain. Key trick: broadcast the
  per-gt row max (and its argmax index) along the free dim BEFORE the
  32x32 stream transpose, so after the transpose every partition holds
  the full per-gt candidate row; the second-stage max and the one-hot
  index select then produce per-partition-broadcast results directly,
  which feed the iota-equality column mask of the cost matrix without
  any partition_broadcast or values_load.
- Per step, two register-offset HWDGE gathers (SP) fetch the matched
  predictions' caption-logit slabs (15 x V/8 floats each), overlapped
  with the serial matching; every 4 steps one ACT sweep computes
  exp + free-dim accumulate -> per-(b,step,pos) partial sum(exp).
- Matched boxes / objectness are recovered post-loop with one-hot
  tensor_tensor_reduce selects from SBUF (no DMAs), then the bbox
  L1/GIoU loss and objectness BCE reduce to per-sample scalars.
- Host: shards caption_logits by vocab, all-reduces the per-core
  partial sumexps, takes log, gathers target-token logits, and combines
  the scalar losses.
"""

import sys

sys.path.insert(0, "/opt/trn_rl_repo")

import numpy as np

import concourse.bacc as bacc
import concourse.mybir as mybir
from concourse.bass import ds
from concourse.tile import TileContext

F32 = mybir.dt.float32
I32 = mybir.dt.int32
U32 = mybir.dt.uint32
Alu = mybir.AluOpType
Act = mybir.ActivationFunctionType

B, N, M, L = 2, 256, 32, 16
LM1 = L - 1  # 15 caption positions
S = M  # greedy steps
NEG = -1.0e9
EPS = 1e-7
ROWS_PER_STEP = B * LM1  # 30 gathered rows per step
STEPS_PER_BATCH = 4
NBATCH = S // STEPS_PER_BATCH  # 8 ACT sweeps over (120, V8)
GP = STEPS_PER_BATCH * ROWS_PER_STEP  # 120


def build_nc(V8: int, num_devices: int = 8):
    """Build the per-core Bass program. V8 = vocab slice width per core."""
    nc = bacc.Bacc(
        "TRN2", target_bir_lowering=False, debug=False, num_devices=num_devices
    )
    SPE = (mybir.EngineType.SP,)
    ACTE = (mybir.EngineType.Activation,)

    cl = nc.dram_tensor("cl", (B * N * L, V8), F32, kind="ExternalInput")
    # pbig: per (b,j) partition, 9 x 256 row segments:
    # [x1n y1n x2n y2n x1 y1 x2 y2 po]
    pbig = nc.dram_tensor("pbig", (64, 9 * N), F32, kind="ExternalInput")
    gb = nc.dram_tensor("gb", (B * M, 4), F32, kind="ExternalInput")
    # gbigT: raw gt coords broadcast along partitions, transposed layout:
    # [p, 32*c + j] = gt_boxes[p//32, j, c]
    gbigT = nc.dram_tensor("gbigT", (64, 4 * M), F32, kind="ExternalInput")
    # cst: host-built constants: cols 0-255 iota, col 256 partition idx mod 32
    cst = nc.dram_tensor("cst", (64, N + 1), F32, kind="ExternalInput")
    out = nc.dram_tensor("out", (128, 16), F32, kind="ExternalOutput")

    # per-sample DRAM view for register-offset caption gathers
    cl2 = cl[:].rearrange("(b n l) v -> b n (l v)", b=B, n=N)  # (2, 256, L*V8)

    with TileContext(nc) as tc:
        with (
            tc.tile_pool(name="cpool", bufs=1) as cp,
            tc.tile_pool(name="gpool", bufs=6) as gp,
            tc.tile_pool(name="dpool", bufs=1) as dp,
        ):
            # ---------- input loads ----------
            pbig_sb = cp.tile([64, 9 * N], F32)
            nc.sync.dma_start(pbig_sb[:], pbig[:])

            def seg(k):
                return pbig_sb[:, k * N : (k + 1) * N]

            gb_sb = cp.tile([64, 4], F32)
            nc.sync.dma_start(gb_sb[:], gb[:])
            gbigT_sb = cp.tile([64, 4 * M], F32)
            nc.sync.dma_start(gbigT_sb[:], gbigT[:])

            ts = nc.vector.tensor_scalar
            tt = nc.vector.tensor_tensor
            ttr = nc.vector.tensor_tensor_reduce

            # ---------- constant tiles (host-supplied iotas) ----------
            cst_sb = cp.tile([64, N + 1], F32)
            nc.sync.dma_start(cst_sb[:], cst[:])
            iota256f = cst_sb[:, 0:N]
            iota32f = cst_sb[:, 0:32]
            iotaPf = cst_sb[:, N : N + 1]
            z32 = cp.tile([64, 32], F32)
            nc.vector.memset(z32[:], 0.0)
            negrow = cp.tile([64, N], F32)
            nc.vector.memset(negrow[:], NEG)

            # ---------- cost matrix build ----------
            # gt cols (64,1)
            gx1n = cp.tile([64, 1], F32)
            gy1n = cp.tile([64, 1], F32)
            gx2n = cp.tile([64, 1], F32)
            gy2n = cp.tile([64, 1], F32)
            tt(gx1n[:], gb_sb[:, 0:1], gb_sb[:, 2:3], op=Alu.min)
            tt(gx2n[:], gb_sb[:, 0:1], gb_sb[:, 2:3], op=Alu.max)
            tt(gy1n[:], gb_sb[:, 1:2], gb_sb[:, 3:4], op=Alu.min)
            tt(gy2n[:], gb_sb[:, 1:2], gb_sb[:, 3:4], op=Alu.max)
            ga2 = cp.tile([64, 1], F32)
            gw = cp.tile([64, 1], F32)
            gh = cp.tile([64, 1], F32)
            tt(gw[:], gx2n[:], gx1n[:], op=Alu.subtract)
            tt(gh[:], gy2n[:], gy1n[:], op=Alu.subtract)
            tt(ga2[:], gw[:], gh[:], op=Alu.mult)

            xi1 = cp.tile([64, N], F32)
            xi2 = cp.tile([64, N], F32)
            xe1 = cp.tile([64, N], F32)
            xe2 = cp.tile([64, N], F32)
            ts(xi1[:], seg(0), gx1n[:], None, op0=Alu.max)
            ts(xi2[:], seg(2), gx2n[:], None, op0=Alu.min)
            ts(xe1[:], seg(0), gx1n[:], None, op0=Alu.min)
            ts(xe2[:], seg(2), gx2n[:], None, op0=Alu.max)
            yi1 = cp.tile([64, N], F32)
            yi2 = cp.tile([64, N], F32)
            ye1 = cp.tile([64, N], F32)
            ye2 = cp.tile([64, N], F32)
            ts(yi1[:], seg(1), gy1n[:], None, op0=Alu.max)
            ts(yi2[:], seg(3), gy2n[:], None, op0=Alu.min)
            ts(ye1[:], seg(1), gy1n[:], None, op0=Alu.min)
            ts(ye2[:], seg(3), gy2n[:], None, op0=Alu.max)

            iw = cp.tile([64, N], F32)
            ih = cp.tile([64, N], F32)
            tt(iw[:], xi2[:], xi1[:], op=Alu.subtract)
            ts(iw[:], iw[:], 0.0, None, op0=Alu.max)
            tt(ih[:], yi2[:], yi1[:], op=Alu.subtract)
            ts(ih[:], ih[:], 0.0, None, op0=Alu.max)
            inter = cp.tile([64, N], F32)
            tt(inter[:], iw[:], ih[:], op=Alu.mult)

            ew = cp.tile([64, N], F32)
            eh = cp.tile([64, N], F32)
            tt(ew[:], xe2[:], xe1[:], op=Alu.subtract)
            tt(eh[:], ye2[:], ye1[:], op=Alu.subtract)
            enc = cp.tile([64, N], F32)
            tt(enc[:], ew[:], eh[:], op=Alu.mult)

            # a1 = (x2n-x1n)*(y2n-y1n); union = a1 + a2 - inter
            a1 = cp.tile([64, N], F32)
            a1h = cp.tile([64, N], F32)
            tt(a1[:], seg(2), seg(0), op=Alu.subtract)
            tt(a1h[:], seg(3), seg(1), op=Alu.subtract)
            tt(a1[:], a1[:], a1h[:], op=Alu.mult)
            union = cp.tile([64, N], F32)
            ts(union[:], a1[:], ga2[:], None, op0=Alu.add)
            tt(union[:], union[:], inter[:], op=Alu.subtract)

            iou = cp.tile([64, N], F32)
            tmp = cp.tile([64, N], F32)
            ts(tmp[:], union[:], EPS, None, op0=Alu.add)
            nc.vector.reciprocal(tmp[:], tmp[:])
            tt(iou[:], inter[:], tmp[:], op=Alu.mult)

            # giou = iou - (enc - union)/(enc + eps)
            giou = cp.tile([64, N], F32)
            tt(giou[:], enc[:], union[:], op=Alu.subtract)
            ts(tmp[:], enc[:], EPS, None, op0=Alu.add)
            nc.vector.reciprocal(tmp[:], tmp[:])
            tt(giou[:], giou[:], tmp[:], op=Alu.mult)
            tt(giou[:], iou[:], giou[:], op=Alu.subtract)

            # l1 from raw comps (segments 4..7)
            l1s = cp.tile([64, N], F32)
            dc = cp.tile([64, N], F32)
            for c in range(4):
                dst = l1s if c == 0 else dc
                ts(dst[:], seg(4 + c), gb_sb[:, c : c + 1], None,
                   op0=Alu.subtract)
                nc.scalar.activation(dst[:], dst[:], Act.Abs)
                if c > 0:
                    tt(l1s[:], l1s[:], dc[:], op=Alu.add)

            # objectness term: sigmoid(po) - 2 (po broadcast = segment 8)
            sig2 = cp.tile([64, N], F32)
            nc.scalar.activation(sig2[:], seg(8), Act.Exp, scale=-1.0)
            ts(sig2[:], sig2[:], 1.0, None, op0=Alu.add)
            nc.vector.reciprocal(sig2[:], sig2[:])
            ts(sig2[:], sig2[:], -2.0, None, op0=Alu.add)

            # ncf = giou - l1 + (sigmoid - 2)  (value to MAXIMIZE)
            ncf = cp.tile([64, N], F32)
            tt(ncf[:], giou[:], l1s[:], op=Alu.subtract)
            tt(ncf[:], ncf[:], sig2[:], op=Alu.add)

            # ---------- greedy matching state ----------
            pk = cp.tile([64, 32], F32)      # top-8 per gt row (cols 0-7)
            ridx = cp.tile([64, 32], U32)    # argmax indices (cols 0-7)
            ridxf = cp.tile([64, 1], F32)
            pk0m = cp.tile([64, 1], F32)
            vstag = cp.tile([64, 32], F32)
            istag = cp.tile([64, 32], F32)
            vstagT = cp.tile([64, 32], F32)
            istagT = cp.tile([64, 32], F32)
            g8 = cp.tile([64, 8], F32)
            gi = cp.tile([64, 8], U32)
            jf = cp.tile([64, 1], F32)
            ohj = cp.tile([64, 32], F32)
            dump32 = cp.tile([64, 32], F32)
            if_ = cp.tile([64, 1], F32)
            pen = cp.tile([64, N], F32)
            ohp = cp.tile([64, 1], F32)
            gmaskP = cp.tile([64, 1], F32)
            nc.vector.memset(gmaskP[:], 0.0)
            pisr = cp.tile([64, 32], F32)
            gjsr = cp.tile([64, 32], F32)
            pisri32 = cp.tile([64, 32], I32)

            outsb = cp.tile([128, 16], F32)
            nc.vector.memset(outsb[:], 0.0)

            # ---------- greedy matching loop ----------
            for s in range(S):
                nc.vector.max(pk[:, 0:8], ncf[:])
                nc.vector.max_index(ridx[:, 0:8], pk[:, 0:8], ncf[:])
                nc.vector.tensor_copy(ridxf[:], ridx[:, 0:1])
                # fold gt-row mask into the stage-2 candidates
                tt(pk0m[:], pk[:, 0:1], gmaskP[:], op=Alu.add)
                # broadcast along free dim so the transpose fills every row
                ts(vstag[:], z32[:], pk0m[:], None, op0=Alu.add)
                ts(istag[:], z32[:], ridxf[:], None, op0=Alu.add)
                nc.vector.transpose(vstagT[:], vstag[:])
                nc.vector.transpose(istagT[:], istag[:])
                # stage 2: winner gt (j) per sample, on every partition
                nc.vector.max(g8[:], vstagT[:])
                nc.vector.max_index(gi[:], g8[:], vstagT[:])
                nc.vector.tensor_copy(jf[:], gi[:, 0:1])
                # one-hot of j along free dim; select i = ridx[j]
                ts(ohj[:], iota32f, jf[:], None, op0=Alu.is_equal)
                tt(dump32[:], istagT[:], ohj[:], op=Alu.mult)
                ts(dump32[:], dump32[:], 0.0, None, op0=Alu.add,
                   op1=Alu.add, accum_out=if_[:])
                # mask gt j for stage-2 of later steps (fused two-scalar ts)
                ts(ohp[:], iotaPf, jf[:], NEG, op0=Alu.is_equal, op1=Alu.mult)
                tt(gmaskP[:], gmaskP[:], ohp[:], op=Alu.add)
                # mask pred column i in ncf, both samples at once
                ts(pen[:], iota256f, if_[:], NEG,
                   op0=Alu.is_equal, op1=Alu.mult)
                tt(ncf[:], ncf[:], pen[:], op=Alu.add)
                # record
                nc.vector.tensor_copy(gjsr[:, s : s + 1], jf[:])
                nc.vector.tensor_copy(pisri32[:, s : s + 1], if_[:])

                i0 = nc.values_load(pisri32[0:1, s : s + 1], engines=SPE,
                                    min_val=0, max_val=N - 1,
                                    skip_runtime_bounds_check=True)
                i1 = nc.values_load(pisri32[32:33, s : s + 1], engines=ACTE,
                                    min_val=0, max_val=N - 1,
                                    skip_runtime_bounds_check=True)
                # caption logit rows of the two matched preds: contiguous
                # (L-1)*V8 slabs fetched with register-offset DMAs (HWDGE).
                g, k = divmod(s, STEPS_PER_BATCH)
                if k == 0:
                    gtile = gp.tile([128, V8], F32, tag="gtile")
                base = k * ROWS_PER_STEP
                nc.sync.dma_start(
                    gtile[base : base + LM1, :],
                    cl2[0, ds(i0, 1), 0 : LM1 * V8])
                nc.scalar.dma_start(
                    gtile[base + LM1 : base + ROWS_PER_STEP, :],
                    cl2[1, ds(i1, 1), 0 : LM1 * V8])
                if k == STEPS_PER_BATCH - 1:
                    dump = dp.tile([128, V8], F32, tag="dump")
                    nc.scalar.activation(dump[0:GP, :], gtile[0:GP, :], Act.Exp,
                                         accum_out=outsb[0:GP, g : g + 1])

            # ---------- post: pis/gjs columns via stream transpose ----------
            nc.vector.tensor_copy(pisr[:], pisri32[:])
            pgT = cp.tile([64, 32], F32)
            ggT = cp.tile([64, 32], F32)
            nc.vector.transpose(pgT[:], pisr[:])
            nc.vector.transpose(ggT[:], gjsr[:])
            # pgT[0:32,0] = pis b0; pgT[32:64,0] = pis b1
            nc.vector.tensor_copy(outsb[0:32, 8:9], pgT[0:32, 0:1])
            nc.vector.tensor_copy(outsb[32:64, 8:9], pgT[32:64, 0:1])
            nc.vector.tensor_copy(outsb[0:32, 9:10], ggT[0:32, 0:1])
            nc.vector.tensor_copy(outsb[32:64, 9:10], ggT[32:64, 0:1])

            # ---------- matched boxes via one-hot selects (no DMA) ----------
            # rows 0-31 = sample A steps, 32-63 = sample B steps
            mp = cp.tile([64, 4], F32)
            mg = cp.tile([64, 4], F32)
            pom = cp.tile([64, 1], F32)
            ohA = cp.tile([64, N], F32)
            dump256 = cp.tile([64, N], F32)
            ts(ohA[:], iota256f, pgT[:, 0:1], None, op0=Alu.is_equal)
            for c in range(4):
                tt(dump256[:], ohA[:], seg(4 + c), op=Alu.mult)
                ts(dump256[:], dump256[:], 0.0, None, op0=Alu.add,
                   op1=Alu.add, accum_out=mp[:, c : c + 1])
            tt(dump256[:], ohA[:], seg(8), op=Alu.mult)
            ts(dump256[:], dump256[:], 0.0, None, op0=Alu.add,
               op1=Alu.add, accum_out=pom[:])
            ohG = cp.tile([64, 32], F32)
            ts(ohG[:], iota32f, ggT[:, 0:1], None, op0=Alu.is_equal)
            for c in range(4):
                tt(dump32[:], ohG[:], gbigT_sb[:, c * M : (c + 1) * M],
                   op=Alu.mult)
                ts(dump32[:], dump32[:], 0.0, None, op0=Alu.add,
                   op1=Alu.add, accum_out=mg[:, c : c + 1])

            # ---------- matched-pair bbox loss ----------
            md = cp.tile([64, 4], F32)
            l1p = cp.tile([64, 1], F32)
            tt(md[:], mp[:], mg[:], op=Alu.subtract)
            nc.scalar.activation(md[:], md[:], Act.Abs, accum_out=l1p[:])

            def col(t, c):
                return t[:, c : c + 1]

            mx1 = cp.tile([64, 1], F32)
            my1 = cp.tile([64, 1], F32)
            mx2 = cp.tile([64, 1], F32)
            my2 = cp.tile([64, 1], F32)
            tt(mx1[:], col(mp, 0), col(mp, 2), op=Alu.min)
            tt(mx2[:], col(mp, 0), col(mp, 2), op=Alu.max)
            tt(my1[:], col(mp, 1), col(mp, 3), op=Alu.min)
            tt(my2[:], col(mp, 1), col(mp, 3), op=Alu.max)
            nx1 = cp.tile([64, 1], F32)
            ny1 = cp.tile([64, 1], F32)
            nx2 = cp.tile([64, 1], F32)
            ny2 = cp.tile([64, 1], F32)
            tt(nx1[:], col(mg, 0), col(mg, 2), op=Alu.min)
            tt(nx2[:], col(mg, 0), col(mg, 2), op=Alu.max)
            tt(ny1[:], col(mg, 1), col(mg, 3), op=Alu.min)
            tt(ny2[:], col(mg, 1), col(mg, 3), op=Alu.max)

            w1 = cp.tile([64, 1], F32)
            w2 = cp.tile([64, 1], F32)
            w3 = cp.tile([64, 1], F32)
            w4 = cp.tile([64, 1], F32)
            tt(w1[:], mx1[:], nx1[:], op=Alu.max)  # xi1
            tt(w2[:], mx2[:], nx2[:], op=Alu.min)  # xi2
            tt(w2[:], w2[:], w1[:], op=Alu.subtract)
            ts(w2[:], w2[:], 0.0, None, op0=Alu.max)  # iw
            tt(w1[:], my1[:], ny1[:], op=Alu.max)
            tt(w3[:], my2[:], ny2[:], op=Alu.min)
            tt(w3[:], w3[:], w1[:], op=Alu.subtract)
            ts(w3[:], w3[:], 0.0, None, op0=Alu.max)  # ih
            minter = cp.tile([64, 1], F32)
            tt(minter[:], w2[:], w3[:], op=Alu.mult)
            tt(w1[:], mx2[:], mx1[:], op=Alu.subtract)
            tt(w2[:], my2[:], my1[:], op=Alu.subtract)
            tt(w1[:], w1[:], w2[:], op=Alu.mult)  # a1
            tt(w2[:], nx2[:], nx1[:], op=Alu.subtract)
            tt(w3[:], ny2[:], ny1[:], op=Alu.subtract)
            tt(w2[:], w2[:], w3[:], op=Alu.mult)  # a2
            munion = cp.tile([64, 1], F32)
            tt(munion[:], w1[:], w2[:], op=Alu.add)
            tt(munion[:], munion[:], minter[:], op=Alu.subtract)
            miou = cp.tile([64, 1], F32)
            ts(w1[:], munion[:], EPS, None, op0=Alu.add)
            nc.vector.reciprocal(w1[:], w1[:])
            tt(miou[:], minter[:], w1[:], op=Alu.mult)
            tt(w1[:], mx1[:], nx1[:], op=Alu.min)
            tt(w2[:], mx2[:], nx2[:], op=Alu.max)
            tt(w2[:], w2[:], w1[:], op=Alu.subtract)  # ew
            tt(w1[:], my1[:], ny1[:], op=Alu.min)
            tt(w3[:], my2[:], ny2[:], op=Alu.max)
            tt(w3[:], w3[:], w1[:], op=Alu.subtract)  # eh
            menc = cp.tile([64, 1], F32)
            tt(menc[:], w2[:], w3[:], op=Alu.mult)
            tt(w1[:], menc[:], munion[:], op=Alu.subtract)
            ts(w2[:], menc[:], EPS, None, op0=Alu.add)
            nc.vector.reciprocal(w2[:], w2[:])
            tt(w1[:], w1[:], w2[:], op=Alu.mult)
            mgiou = cp.tile([64, 1], F32)
            tt(mgiou[:], miou[:], w1[:], op=Alu.subtract)
            ts(w4[:], mgiou[:], -1.0, 1.0, op0=Alu.mult, op1=Alu.add)  # 1-giou

            # per-sample sums: transpose each (64,1) vector and accumulate
            # rows 0 / 32 separately.
            sums3 = cp.tile([64, 3], F32)  # col 0=l1, 1=1-g, 2=po; rows 0/32
            for ci, vec in enumerate((l1p[:], w4[:], pom[:])):
                pkx = cp.tile([64, 32], F32, tag="pkx")
                nc.vector.memset(pkx[:], 0.0)
                nc.vector.tensor_copy(pkx[:, 0:1], vec)
                pkxT = cp.tile([64, 32], F32, tag="pkxT")
                nc.vector.transpose(pkxT[:], pkx[:])
                ts(pkxT[0:1, :], pkxT[0:1, :], 0.0, None, op0=Alu.add,
                   op1=Alu.add, accum_out=sums3[0:1, ci : ci + 1])
                ts(pkxT[32:33, :], pkxT[32:33, :], 0.0, None, op0=Alu.add,
                   op1=Alu.add, accum_out=sums3[32:33, ci : ci + 1])

            # objectness base: relu(po) + ln(1+exp(-|po|)) on the broadcast
            # po slab (seg 8); rows 0 / 32 give the per-sample rowsums.
            relu = cp.tile([64, N], F32)
            abspo = cp.tile([64, N], F32)
            sp = cp.tile([64, N], F32)
            basesum = cp.tile([64, 1], F32)
            ts(relu[:], seg(8), 0.0, None, op0=Alu.max)
            nc.scalar.activation(abspo[:], seg(8), Act.Abs)
            nc.scalar.activation(sp[:], abspo[:], Act.Exp, scale=-1.0)
            ts(sp[:], sp[:], 1.0, None, op0=Alu.add)
            nc.scalar.activation(sp[:], sp[:], Act.Ln)
            tt(relu[:], relu[:], sp[:], op=Alu.add)
            ts(relu[:], relu[:], 0.0, None, op0=Alu.add, op1=Alu.add,
               accum_out=basesum[:])

            # bbox_b = clip(l1sum/128 + clip(gsum/32, 0, 2), 0)
            # obj_b = clip((basesum - pomsum)/256, 0)
            # per-sample results at rows 0 and 32 of outsb cols 10/11.
            b1t = cp.tile([64, 1], F32)
            b2t = cp.tile([64, 1], F32)
            obt = cp.tile([64, 1], F32)
            for b in range(2):
                r = 32 * b
                bb = slice(r, r + 1)
                ts(b1t[bb], sums3[bb, 0:1], 1.0 / 128.0, None, op0=Alu.mult)
                ts(b2t[bb], sums3[bb, 1:2], 1.0 / 32.0, None, op0=Alu.mult)
                ts(b2t[bb], b2t[bb], 0.0, 2.0, op0=Alu.max, op1=Alu.min)
                tt(b1t[bb], b1t[bb], b2t[bb], op=Alu.add)
                ts(b1t[bb], b1t[bb], 0.0, None, op0=Alu.max)
                tt(obt[bb], basesum[bb], sums3[bb, 2:3], op=Alu.subtract)
                ts(obt[bb], obt[bb], 1.0 / 256.0, 0.0, op0=Alu.mult, op1=Alu.max)
                nc.vector.tensor_copy(outsb[bb, 10:11], b1t[bb])
                nc.vector.tensor_copy(outsb[bb, 11:12], obt[bb])

            nc.sync.dma_start(out[:], outsb[:])

    nc.compile()
    return nc


# ---------------- host side ----------------

def shard_inputs(pred_boxes, pred_objectness, caption_logits, gt_boxes, V8, NC=8):
    pbf = pred_boxes.astype(np.float32)
    x1n = np.minimum(pbf[..., 0], pbf[..., 2])
    y1n = np.minimum(pbf[..., 1], pbf[..., 3])
    x2n = np.maximum(pbf[..., 0], pbf[..., 2])
    y2n = np.maximum(pbf[..., 1], pbf[..., 3])
    rows = np.stack(
        [x1n, y1n, x2n, y2n, pbf[..., 0], pbf[..., 1], pbf[..., 2], pbf[..., 3],
         pred_objectness.astype(np.float32)], axis=1)  # (B, 9, N)
    pbig = np.broadcast_to(rows[:, None, :, :], (B, M, 9, N)).reshape(64, 9 * N)
    pbig = np.ascontiguousarray(pbig)
    gb = np.ascontiguousarray(gt_boxes.reshape(B * M, 4).astype(np.float32))
    gbf = gt_boxes.astype(np.float32)  # (B, M, 4)
    gbigT = np.zeros((64, 4 * M), np.float32)
    for b in range(B):
        for c in range(4):
            gbigT[32 * b : 32 * b + 32, c * M : (c + 1) * M] = gbf[b, :, c][None, :]
    cstv = np.zeros((64, N + 1), np.float32)
    cstv[:, 0:N] = np.arange(N, dtype=np.float32)[None, :]
    cstv[:, N] = (np.arange(64) % 32).astype(np.float32)
    clv = caption_logits.reshape(B * N * L, NC, V8)
    in_maps = []
    for c in range(NC):
        in_maps.append({
            "cl": np.ascontiguousarray(clv[:, c, :]).astype(np.float32, copy=False),
            "pbig": pbig, "gb": gb, "gbigT": gbigT, "cst": cstv,
        })
    return in_maps


def combine(results, caption_logits, gt_tokens, V8, NC=8):
    """results: list of per-core 'out' arrays (128,16)."""
    out0 = results[0]
    sums = np.zeros((GP, NBATCH), np.float64)
    for c in range(NC):
        sums += results[c][0:GP, 0:NBATCH].astype(np.float64)
    lse = np.log(sums)  # (120, 8): row p = k*30 + b*15 + l, col g; step = 4g+k
    lse_bsl = (
        lse.reshape(STEPS_PER_BATCH, B, LM1, NBATCH)
        .transpose(1, 3, 0, 2)
        .reshape(B, S, LM1)
    )
    pis = out0[0:64, 8].astype(np.int64).reshape(2, 32)
    gjs = out0[0:64, 9].astype(np.int64).reshape(2, 32)
    tok = np.asarray(gt_tokens).astype(np.int64)

    bidx = np.arange(B)[:, None, None]
    lidx = np.arange(LM1)[None, None, :]
    tgt = tok[bidx, gjs[:, :, None], lidx + 1]  # (B, S, LM1)
    tlog = caption_logits[bidx, pis[:, :, None], lidx, tgt].astype(np.float64)
    ce = (lse_bsl - tlog).mean(axis=2)  # (B, S)
    cap = np.clip(np.clip(ce, 0.0, None).mean(axis=1), 0.0, None)  # (B,)
    bbox = out0[[0, 32], 10].astype(np.float64)
    obj = out0[[0, 32], 11].astype(np.float64)
    total = max((5.0 * bbox + 0.1 * cap + obj).mean(), 0.0)
    comps = [5.0 * bbox.mean(), 0.1 * cap.mean(), obj.mean()]
    return np.array([total] + comps, np.float32)


# ---------------- entry points ----------------

V8_FULL = 4000
NC_CORES = 8
_CACHE = {}


def get_nc(V8=V8_FULL):
    key = V8
    if key not in _CACHE:
        _CACHE[key] = build_nc(V8, num_devices=NC_CORES)
    return _CACHE[key]


def run_device(in_maps, V8=V8_FULL, trace=False, **kw):
    from concourse.bass_utils import run_bass_kernel_spmd

    nc = get_nc(V8)
    return run_bass_kernel_spmd(
        nc, in_maps, core_ids=list(range(NC_CORES)), trace=trace, **kw)


def kernel(pred_boxes, pred_objectness, caption_logits, gt_boxes, gt_tokens):
    pred_boxes = np.asarray(pred_boxes, np.float32)
    pred_objectness = np.asarray(pred_objectness, np.float32)
    caption_logits = np.asarray(caption_logits, np.float32)
    gt_boxes = np.asarray(gt_boxes, np.float32)
    in_maps = shard_inputs(
        pred_boxes, pred_objectness, caption_logits, gt_boxes, V8_FULL, NC_CORES)
    res = run_device(in_maps)
    outs = [r["out"] for r in res.results]
    return combine(outs, caption_logits, gt_tokens, V8_FULL, NC_CORES)


# revision 8
# speedup vs baseline: 2.0470x; 1.0640x over previous
"""DetectionLoss Bass kernel for TRN2, 8-core SPMD (v2).

Strategy (identical program on all 8 cores; inputs differ only in the
vocab slice of caption_logits):
- Build the (64,256) fused cost matrix (both samples stacked on the
  partition dim) from boxes + objectness.
- 32-step greedy matching entirely on DVE with zero registers and zero
  cross-engine hops on the critical chain. Key trick: broadcast the
  per-gt row max (and its argmax index) along the free dim BEFORE the
  32x32 stream transpose, so after the transpose every partition holds
  the full per-gt candidate row; the second-stage max and the one-hot
  index select then produce per-partition-broadcast results directly,
  which feed the iota-equality column mask of the cost matrix without
  any partition_broadcast or values_load.
- Per step, two register-offset HWDGE gathers (SP) fetch the matched
  predictions' caption-logit slabs (15 x V/8 floats each), overlapped
  with the serial matching; every 4 steps one ACT sweep computes
  exp + free-dim accumulate -> per-(b,step,pos) partial sum(exp).
- Matched boxes / objectness are recovered post-loop with one-hot
  tensor_tensor_reduce selects from SBUF (no DMAs), then the bbox
  L1/GIoU loss and objectness BCE reduce to per-sample scalars.
- Host: shards caption_logits by vocab, all-reduces the per-core
  partial sumexps, takes log, gathers target-token logits, and combines
  the scalar losses.
"""

import sys

sys.path.insert(0, "/opt/trn_rl_repo")

import numpy as np

import concourse.bacc as bacc
import concourse.mybir as mybir
from concourse.bass import ds
from concourse.tile import TileContext

F32 = mybir.dt.float32
I32 = mybir.dt.int32
U32 = mybir.dt.uint32
Alu = mybir.AluOpType
Act = mybir.ActivationFunctionType

B, N, M, L = 2, 256, 32, 16
LM1 = L - 1  # 15 caption positions
S = M  # greedy steps
NEG = -1.0e9
EPS = 1e-7
ROWS_PER_STEP = B * LM1  # 30 gathered rows per step
STEPS_PER_BATCH = 4
NBATCH = S // STEPS_PER_BATCH  # 8 ACT sweeps over (120, V8)
GP = STEPS_PER_BATCH * ROWS_PER_STEP  # 120


def build_nc(V8: int, num_devices: int = 8):
    """Build the per-core Bass program. V8 = vocab slice width per core."""
    nc = bacc.Bacc(
        "TRN2", target_bir_lowering=False, debug=False, num_devices=num_devices
    )
    SPE = (mybir.EngineType.SP,)
    ACTE = (mybir.EngineType.Activation,)

    cl = nc.dram_tensor("cl", (B * N * L, V8), F32, kind="ExternalInput")
    # pbig: per (b,j) partition, 9 x 256 row segments:
    # [x1n y1n x2n y2n x1 y1 x2 y2 po]
    pbig = nc.dram_tensor("pbig", (64, 9 * N), F32, kind="ExternalInput")
    gb = nc.dram_tensor("gb", (B * M, 4), F32, kind="ExternalInput")
    # gbigT: raw gt coords broadcast along partitions, transposed layout:
    # [p, 32*c + j] = gt_boxes[p//32, j, c]
    gbigT = nc.dram_tensor("gbigT", (64, 4 * M), F32, kind="ExternalInput")
    # cst: host-built constants: cols 0-255 iota, col 256 partition idx mod 32
    cst = nc.dram_tensor("cst", (64, N + 1), F32, kind="ExternalInput")
    out = nc.dram_tensor("out", (128, 16), F32, kind="ExternalOutput")

    # per-sample DRAM view for register-offset caption gathers
    cl2 = cl[:].rearrange("(b n l) v -> b n (l v)", b=B, n=N)  # (2, 256, L*V8)

    with TileContext(nc) as tc:
        with (
            tc.tile_pool(name="cpool", bufs=1) as cp,
            tc.tile_pool(name="gpool", bufs=8) as gp,
            tc.tile_pool(name="dpool", bufs=1) as dp,
        ):
            # ---------- input loads ----------
            pbig_sb = cp.tile([64, 9 * N], F32)
            nc.sync.dma_start(pbig_sb[:], pbig[:])

            def seg(k):
                return pbig_sb[:, k * N : (k + 1) * N]

            gb_sb = cp.tile([64, 4], F32)
            nc.sync.dma_start(gb_sb[:], gb[:])
            gbigT_sb = cp.tile([64, 4 * M], F32)
            nc.sync.dma_start(gbigT_sb[:], gbigT[:])

            ts = nc.vector.tensor_scalar
            tt = nc.vector.tensor_tensor
            ttr = nc.vector.tensor_tensor_reduce

            # ---------- constant tiles (host-supplied iotas) ----------
            cst_sb = cp.tile([64, N + 1], F32)
            nc.sync.dma_start(cst_sb[:], cst[:])
            iota256f = cst_sb[:, 0:N]
            iota32f = cst_sb[:, 0:32]
            iotaPf = cst_sb[:, N : N + 1]
            z32 = cp.tile([64, 32], F32)
            nc.vector.memset(z32[:], 0.0)
            negrow = cp.tile([64, N], F32)
            nc.vector.memset(negrow[:], NEG)

            # ---------- cost matrix build ----------
            # gt cols (64,1)
            gx1n = cp.tile([64, 1], F32)
            gy1n = cp.tile([64, 1], F32)
            gx2n = cp.tile([64, 1], F32)
            gy2n = cp.tile([64, 1], F32)
            tt(gx1n[:], gb_sb[:, 0:1], gb_sb[:, 2:3], op=Alu.min)
            tt(gx2n[:], gb_sb[:, 0:1], gb_sb[:, 2:3], op=Alu.max)
            tt(gy1n[:], gb_sb[:, 1:2], gb_sb[:, 3:4], op=Alu.min)
            tt(gy2n[:], gb_sb[:, 1:2], gb_sb[:, 3:4], op=Alu.max)
            ga2 = cp.tile([64, 1], F32)
            gw = cp.tile([64, 1], F32)
            gh = cp.tile([64, 1], F32)
            tt(gw[:], gx2n[:], gx1n[:], op=Alu.subtract)
            tt(gh[:], gy2n[:], gy1n[:], op=Alu.subtract)
            tt(ga2[:], gw[:], gh[:], op=Alu.mult)

            xi1 = cp.tile([64, N], F32)
            xi2 = cp.tile([64, N], F32)
            xe1 = cp.tile([64, N], F32)
            xe2 = cp.tile([64, N], F32)
            ts(xi1[:], seg(0), gx1n[:], None, op0=Alu.max)
            ts(xi2[:], seg(2), gx2n[:], None, op0=Alu.min)
            ts(xe1[:], seg(0), gx1n[:], None, op0=Alu.min)
            ts(xe2[:], seg(2), gx2n[:], None, op0=Alu.max)
            yi1 = cp.tile([64, N], F32)
            yi2 = cp.tile([64, N], F32)
            ye1 = cp.tile([64, N], F32)
            ye2 = cp.tile([64, N], F32)
            ts(yi1[:], seg(1), gy1n[:], None, op0=Alu.max)
            ts(yi2[:], seg(3), gy2n[:], None, op0=Alu.min)
            ts(ye1[:], seg(1), gy1n[:], None, op0=Alu.min)
            ts(ye2[:], seg(3), gy2n[:], None, op0=Alu.max)

            iw = cp.tile([64, N], F32)
            ih = cp.tile([64, N], F32)
            tt(iw[:], xi2[:], xi1[:], op=Alu.subtract)
            ts(iw[:], iw[:], 0.0, None, op0=Alu.max)
            tt(ih[:], yi2[:], yi1[:], op=Alu.subtract)
            ts(ih[:], ih[:], 0.0, None, op0=Alu.max)
            inter = cp.tile([64, N], F32)
            tt(inter[:], iw[:], ih[:], op=Alu.mult)

            ew = cp.tile([64, N], F32)
            eh = cp.tile([64, N], F32)
            tt(ew[:], xe2[:], xe1[:], op=Alu.subtract)
            tt(eh[:], ye2[:], ye1[:], op=Alu.subtract)
            enc = cp.tile([64, N], F32)
            tt(enc[:], ew[:], eh[:], op=Alu.mult)

            # a1 = (x2n-x1n)*(y2n-y1n); union = a1 + a2 - inter
            a1 = cp.tile([64, N], F32)
            a1h = cp.tile([64, N], F32)
            tt(a1[:], seg(2), seg(0), op=Alu.subtract)
            tt(a1h[:], seg(3), seg(1), op=Alu.subtract)
            tt(a1[:], a1[:], a1h[:], op=Alu.mult)
            union = cp.tile([64, N], F32)
            ts(union[:], a1[:], ga2[:], None, op0=Alu.add)
            tt(union[:], union[:], inter[:], op=Alu.subtract)

            iou = cp.tile([64, N], F32)
            tmp = cp.tile([64, N], F32)
            ts(tmp[:], union[:], EPS, None, op0=Alu.add)
            nc.vector.reciprocal(tmp[:], tmp[:])
            tt(iou[:], inter[:], tmp[:], op=Alu.mult)

            # giou = iou - (enc - union)/(enc + eps)
            giou = cp.tile([64, N], F32)
            tt(giou[:], enc[:], union[:], op=Alu.subtract)
            ts(tmp[:], enc[:], EPS, None, op0=Alu.add)
            nc.vector.reciprocal(tmp[:], tmp[:])
            tt(giou[:], giou[:], tmp[:], op=Alu.mult)
            tt(giou[:], iou[:], giou[:], op=Alu.subtract)

            # l1 from raw comps (segments 4..7)
            l1s = cp.tile([64, N], F32)
            dc = cp.tile([64, N], F32)
            dn = cp.tile([64, N], F32)
            for c in range(4):
                dst = l1s if c == 0 else dc
                ts(dst[:], seg(4 + c), gb_sb[:, c : c + 1], None,
                   op0=Alu.subtract)
                ts(dn[:], dst[:], -1.0, None, op0=Alu.mult)
                tt(dst[:], dst[:], dn[:], op=Alu.max)
                if c > 0:
                    tt(l1s[:], l1s[:], dc[:], op=Alu.add)

            # objectness term: sigmoid(po) - 2 (po broadcast = segment 8)
            sig2 = cp.tile([64, N], F32)
            nc.scalar.activation(sig2[:], seg(8), Act.Exp, scale=-1.0)
            ts(sig2[:], sig2[:], 1.0, None, op0=Alu.add)
            nc.vector.reciprocal(sig2[:], sig2[:])
            ts(sig2[:], sig2[:], -2.0, None, op0=Alu.add)

            # ncf = giou - l1 + (sigmoid - 2)  (value to MAXIMIZE)
            ncf = cp.tile([64, N], F32)
            tt(ncf[:], giou[:], l1s[:], op=Alu.subtract)
            tt(ncf[:], ncf[:], sig2[:], op=Alu.add)

            # ---------- greedy matching state ----------
            pk = cp.tile([64, 32], F32)      # top-8 per gt row (cols 0-7)
            ridx = cp.tile([64, 32], U32)    # argmax indices (cols 0-7)
            ridxf = cp.tile([64, 1], F32)
            pk0m = cp.tile([64, 1], F32)
            vstag = cp.tile([64, 32], F32)
            istag = cp.tile([64, 32], F32)
            vstagT = cp.tile([64, 32], F32)
            istagT = cp.tile([64, 32], F32)
            g8 = cp.tile([64, 8], F32)
            gi = cp.tile([64, 8], U32)
            jf = cp.tile([64, 1], F32)
            ohj = cp.tile([64, 32], F32)
            dump32 = cp.tile([64, 32], F32)
            if_ = cp.tile([64, 1], F32)
            pen = cp.tile([64, N], F32)
            ohp = cp.tile([64, 1], F32)
            gmaskP = cp.tile([64, 1], F32)
            nc.vector.memset(gmaskP[:], 0.0)
            pisr = cp.tile([64, 32], F32)
            gjsr = cp.tile([64, 32], F32)
            pisri32 = cp.tile([64, 32], I32)

            outsb = cp.tile([128, 16], F32)
            nc.vector.memset(outsb[:], 0.0)

            # ---------- greedy matching loop ----------
            for s in range(S):
                nc.vector.max(pk[:, 0:8], ncf[:])
                nc.vector.max_index(ridx[:, 0:8], pk[:, 0:8], ncf[:])
                nc.vector.tensor_copy(ridxf[:], ridx[:, 0:1])
                # fold gt-row mask into the stage-2 candidates
                tt(pk0m[:], pk[:, 0:1], gmaskP[:], op=Alu.add)
                # broadcast along free dim so the transpose fills every row
                ts(vstag[:], z32[:], pk0m[:], None, op0=Alu.add)
                ts(istag[:], z32[:], ridxf[:], None, op0=Alu.add)
                nc.vector.transpose(vstagT[:], vstag[:])
                nc.vector.transpose(istagT[:], istag[:])
                # stage 2: winner gt (j) per sample, on every partition
                nc.vector.max(g8[:], vstagT[:])
                nc.vector.max_index(gi[:], g8[:], vstagT[:])
                nc.vector.tensor_copy(jf[:], gi[:, 0:1])
                # one-hot of j along free dim; select i = ridx[j]
                ts(ohj[:], iota32f, jf[:], None, op0=Alu.is_equal)
                tt(dump32[:], istagT[:], ohj[:], op=Alu.mult)
                ts(dump32[:], dump32[:], 0.0, None, op0=Alu.add,
                   op1=Alu.add, accum_out=if_[:])
                # mask gt j for stage-2 of later steps (fused two-scalar ts)
                ts(ohp[:], iotaPf, jf[:], NEG, op0=Alu.is_equal, op1=Alu.mult)
                tt(gmaskP[:], gmaskP[:], ohp[:], op=Alu.add)
                # mask pred column i in ncf, both samples at once
                ts(pen[:], iota256f, if_[:], NEG,
                   op0=Alu.is_equal, op1=Alu.mult)
                tt(ncf[:], ncf[:], pen[:], op=Alu.add)
                # record
                nc.vector.tensor_copy(gjsr[:, s : s + 1], jf[:])
                nc.vector.tensor_copy(pisri32[:, s : s + 1], if_[:])

                i0 = nc.values_load(pisri32[0:1, s : s + 1], engines=SPE,
                                    min_val=0, max_val=N - 1,
                                    skip_runtime_bounds_check=True)
                i1 = nc.values_load(pisri32[32:33, s : s + 1], engines=SPE,
                                    min_val=0, max_val=N - 1,
                                    skip_runtime_bounds_check=True)
                # caption logit rows of the two matched preds: contiguous
                # (L-1)*V8 slabs fetched with register-offset DMAs (HWDGE).
                g, k = divmod(s, STEPS_PER_BATCH)
                if k == 0:
                    gtile = gp.tile([128, V8], F32, tag="gtile")
                base = k * ROWS_PER_STEP
                nc.sync.dma_start(
                    gtile[base : base + LM1, :],
                    cl2[0, ds(i0, 1), 0 : LM1 * V8])
                nc.sync.dma_start(
                    gtile[base + LM1 : base + ROWS_PER_STEP, :],
                    cl2[1, ds(i1, 1), 0 : LM1 * V8])
                if k == STEPS_PER_BATCH - 1:
                    dump = dp.tile([128, V8], F32, tag="dump")
                    nc.scalar.activation(dump[0:GP, :], gtile[0:GP, :], Act.Exp,
                                         accum_out=outsb[0:GP, g : g + 1])

            # ---------- post: pis/gjs columns via stream transpose ----------
            nc.vector.tensor_copy(pisr[:], pisri32[:])
            pgT = cp.tile([64, 32], F32)
            ggT = cp.tile([64, 32], F32)
            nc.vector.transpose(pgT[:], pisr[:])
            nc.vector.transpose(ggT[:], gjsr[:])
            # pgT[0:32,0] = pis b0; pgT[32:64,0] = pis b1
            nc.vector.tensor_copy(outsb[0:32, 8:9], pgT[0:32, 0:1])
            nc.vector.tensor_copy(outsb[32:64, 8:9], pgT[32:64, 0:1])
            nc.vector.tensor_copy(outsb[0:32, 9:10], ggT[0:32, 0:1])
            nc.vector.tensor_copy(outsb[32:64, 9:10], ggT[32:64, 0:1])

            # ---------- matched boxes via one-hot selects (no DMA) ----------
            # rows 0-31 = sample A steps, 32-63 = sample B steps
            mp = cp.tile([64, 4], F32)
            mg = cp.tile([64, 4], F32)
            pom = cp.tile([64, 1], F32)
            ohA = cp.tile([64, N], F32)
            dump256 = cp.tile([64, N], F32)
            ts(ohA[:], iota256f, pgT[:, 0:1], None, op0=Alu.is_equal)
            for c in range(4):
                tt(dump256[:], ohA[:], seg(4 + c), op=Alu.mult)
                ts(dump256[:], dump256[:], 0.0, None, op0=Alu.add,
                   op1=Alu.add, accum_out=mp[:, c : c + 1])
            tt(dump256[:], ohA[:], seg(8), op=Alu.mult)
            ts(dump256[:], dump256[:], 0.0, None, op0=Alu.add,
               op1=Alu.add, accum_out=pom[:])
            ohG = cp.tile([64, 32], F32)
            ts(ohG[:], iota32f, ggT[:, 0:1], None, op0=Alu.is_equal)
            for c in range(4):
                tt(dump32[:], ohG[:], gbigT_sb[:, c * M : (c + 1) * M],
                   op=Alu.mult)
                ts(dump32[:], dump32[:], 0.0, None, op0=Alu.add,
                   op1=Alu.add, accum_out=mg[:, c : c + 1])

            # ---------- matched-pair bbox loss ----------
            md = cp.tile([64, 4], F32)
            l1p = cp.tile([64, 1], F32)
            tt(md[:], mp[:], mg[:], op=Alu.subtract)
            nc.scalar.activation(md[:], md[:], Act.Abs, accum_out=l1p[:])

            def col(t, c):
                return t[:, c : c + 1]

            mx1 = cp.tile([64, 1], F32)
            my1 = cp.tile([64, 1], F32)
            mx2 = cp.tile([64, 1], F32)
            my2 = cp.tile([64, 1], F32)
            tt(mx1[:], col(mp, 0), col(mp, 2), op=Alu.min)
            tt(mx2[:], col(mp, 0), col(mp, 2), op=Alu.max)
            tt(my1[:], col(mp, 1), col(mp, 3), op=Alu.min)
            tt(my2[:], col(mp, 1), col(mp, 3), op=Alu.max)
            nx1 = cp.tile([64, 1], F32)
            ny1 = cp.tile([64, 1], F32)
            nx2 = cp.tile([64, 1], F32)
            ny2 = cp.tile([64, 1], F32)
            tt(nx1[:], col(mg, 0), col(mg, 2), op=Alu.min)
            tt(nx2[:], col(mg, 0), col(mg, 2), op=Alu.max)
            tt(ny1[:], col(mg, 1), col(mg, 3), op=Alu.min)
            tt(ny2[:], col(mg, 1), col(mg, 3), op=Alu.max)

            w1 = cp.tile([64, 1], F32)
            w2 = cp.tile([64, 1], F32)
            w3 = cp.tile([64, 1], F32)
            w4 = cp.tile([64, 1], F32)
            tt(w1[:], mx1[:], nx1[:], op=Alu.max)  # xi1
            tt(w2[:], mx2[:], nx2[:], op=Alu.min)  # xi2
            tt(w2[:], w2[:], w1[:], op=Alu.subtract)
            ts(w2[:], w2[:], 0.0, None, op0=Alu.max)  # iw
            tt(w1[:], my1[:], ny1[:], op=Alu.max)
            tt(w3[:], my2[:], ny2[:], op=Alu.min)
            tt(w3[:], w3[:], w1[:], op=Alu.subtract)
            ts(w3[:], w3[:], 0.0, None, op0=Alu.max)  # ih
            minter = cp.tile([64, 1], F32)
            tt(minter[:], w2[:], w3[:], op=Alu.mult)
            tt(w1[:], mx2[:], mx1[:], op=Alu.subtract)
            tt(w2[:], my2[:], my1[:], op=Alu.subtract)
            tt(w1[:], w1[:], w2[:], op=Alu.mult)  # a1
            tt(w2[:], nx2[:], nx1[:], op=Alu.subtract)
            tt(w3[:], ny2[:], ny1[:], op=Alu.subtract)
            tt(w2[:], w2[:], w3[:], op=Alu.mult)  # a2
            munion = cp.tile([64, 1], F32)
            tt(munion[:], w1[:], w2[:], op=Alu.add)
            tt(munion[:], munion[:], minter[:], op=Alu.subtract)
            miou = cp.tile([64, 1], F32)
            ts(w1[:], munion[:], EPS, None, op0=Alu.add)
            nc.vector.reciprocal(w1[:], w1[:])
            tt(miou[:], minter[:], w1[:], op=Alu.mult)
            tt(w1[:], mx1[:], nx1[:], op=Alu.min)
            tt(w2[:], mx2[:], nx2[:], op=Alu.max)
            tt(w2[:], w2[:], w1[:], op=Alu.subtract)  # ew
            tt(w1[:], my1[:], ny1[:], op=Alu.min)
            tt(w3[:], my2[:], ny2[:], op=Alu.max)
            tt(w3[:], w3[:], w1[:], op=Alu.subtract)  # eh
            menc = cp.tile([64, 1], F32)
            tt(menc[:], w2[:], w3[:], op=Alu.mult)
            tt(w1[:], menc[:], munion[:], op=Alu.subtract)
            ts(w2[:], menc[:], EPS, None, op0=Alu.add)
            nc.vector.reciprocal(w2[:], w2[:])
            tt(w1[:], w1[:], w2[:], op=Alu.mult)
            mgiou = cp.tile([64, 1], F32)
            tt(mgiou[:], miou[:], w1[:], op=Alu.subtract)
            ts(w4[:], mgiou[:], -1.0, 1.0, op0=Alu.mult, op1=Alu.add)  # 1-giou

            # per-sample sums: transpose each (64,1) vector and accumulate
            # rows 0 / 32 separately.
            sums3 = cp.tile([64, 3], F32)  # col 0=l1, 1=1-g, 2=po; rows 0/32
            for ci, vec in enumerate((l1p[:], w4[:], pom[:])):
                pkx = cp.tile([64, 32], F32, tag="pkx")
                nc.vector.memset(pkx[:], 0.0)
                nc.vector.tensor_copy(pkx[:, 0:1], vec)
                pkxT = cp.tile([64, 32], F32, tag="pkxT")
                nc.vector.transpose(pkxT[:], pkx[:])
                ts(pkxT[0:1, :], pkxT[0:1, :], 0.0, None, op0=Alu.add,
                   op1=Alu.add, accum_out=sums3[0:1, ci : ci + 1])
                ts(pkxT[32:33, :], pkxT[32:33, :], 0.0, None, op0=Alu.add,
                   op1=Alu.add, accum_out=sums3[32:33, ci : ci + 1])

            # objectness base: relu(po) + ln(1+exp(-|po|)) on the broadcast
            # po slab (seg 8); rows 0 / 32 give the per-sample rowsums.
            relu = cp.tile([64, N], F32)
            abspo = cp.tile([64, N], F32)
            sp = cp.tile([64, N], F32)
            basesum = cp.tile([64, 1], F32)
            ts(relu[:], seg(8), 0.0, None, op0=Alu.max)
            nc.scalar.activation(abspo[:], seg(8), Act.Abs)
            nc.scalar.activation(sp[:], abspo[:], Act.Exp, scale=-1.0)
            ts(sp[:], sp[:], 1.0, None, op0=Alu.add)
            nc.scalar.activation(sp[:], sp[:], Act.Ln)
            tt(relu[:], relu[:], sp[:], op=Alu.add)
            ts(relu[:], relu[:], 0.0, None, op0=Alu.add, op1=Alu.add,
               accum_out=basesum[:])

            # bbox_b = clip(l1sum/128 + clip(gsum/32, 0, 2), 0)
            # obj_b = clip((basesum - pomsum)/256, 0)
            # per-sample results at rows 0 and 32 of outsb cols 10/11.
            b1t = cp.tile([64, 1], F32)
            b2t = cp.tile([64, 1], F32)
            obt = cp.tile([64, 1], F32)
            for b in range(2):
                r = 32 * b
                bb = slice(r, r + 1)
                ts(b1t[bb], sums3[bb, 0:1], 1.0 / 128.0, None, op0=Alu.mult)
                ts(b2t[bb], sums3[bb, 1:2], 1.0 / 32.0, None, op0=Alu.mult)
                ts(b2t[bb], b2t[bb], 0.0, 2.0, op0=Alu.max, op1=Alu.min)
                tt(b1t[bb], b1t[bb], b2t[bb], op=Alu.add)
                ts(b1t[bb], b1t[bb], 0.0, None, op0=Alu.max)
                tt(obt[bb], basesum[bb], sums3[bb, 2:3], op=Alu.subtract)
                ts(obt[bb], obt[bb], 1.0 / 256.0, 0.0, op0=Alu.mult, op1=Alu.max)
                nc.vector.tensor_copy(outsb[bb, 10:11], b1t[bb])
                nc.vector.tensor_copy(outsb[bb, 11:12], obt[bb])

            nc.sync.dma_start(out[:], outsb[:])

    nc.compile()
    return nc


# ---------------- host side ----------------

def shard_inputs(pred_boxes, pred_objectness, caption_logits, gt_boxes, V8, NC=8):
    pbf = pred_boxes.astype(np.float32)
    x1n = np.minimum(pbf[..., 0], pbf[..., 2])
    y1n = np.minimum(pbf[..., 1], pbf[..., 3])
    x2n = np.maximum(pbf[..., 0], pbf[..., 2])
    y2n = np.maximum(pbf[..., 1], pbf[..., 3])
    rows = np.stack(
        [x1n, y1n, x2n, y2n, pbf[..., 0], pbf[..., 1], pbf[..., 2], pbf[..., 3],
         pred_objectness.astype(np.float32)], axis=1)  # (B, 9, N)
    pbig = np.broadcast_to(rows[:, None, :, :], (B, M, 9, N)).reshape(64, 9 * N)
    pbig = np.ascontiguousarray(pbig)
    gb = np.ascontiguousarray(gt_boxes.reshape(B * M, 4).astype(np.float32))
    gbf = gt_boxes.astype(np.float32)  # (B, M, 4)
    gbigT = np.zeros((64, 4 * M), np.float32)
    for b in range(B):
        for c in range(4):
            gbigT[32 * b : 32 * b + 32, c * M : (c + 1) * M] = gbf[b, :, c][None, :]
    cstv = np.zeros((64, N + 1), np.float32)
    cstv[:, 0:N] = np.arange(N, dtype=np.float32)[None, :]
    cstv[:, N] = (np.arange(64) % 32).astype(np.float32)
    clv = caption_logits.reshape(B * N * L, NC, V8)
    in_maps = []
    for c in range(NC):
        in_maps.append({
            "cl": np.ascontiguousarray(clv[:, c, :]).astype(np.float32, copy=False),
            "pbig": pbig, "gb": gb, "gbigT": gbigT, "cst": cstv,
        })
    return in_maps


def combine(results, caption_logits, gt_tokens, V8, NC=8):
    """results: list of per-core 'out' arrays (128,16)."""
    out0 = results[0]
    sums = np.zeros((GP, NBATCH), np.float64)
    for c in range(NC):
        sums += results[c][0:GP, 0:NBATCH].astype(np.float64)
    lse = np.log(sums)  # (120, 8): row p = k*30 + b*15 + l, col g; step = 4g+k
    lse_bsl = (
        lse.reshape(STEPS_PER_BATCH, B, LM1, NBATCH)
        .transpose(1, 3, 0, 2)
        .reshape(B, S, LM1)
    )
    pis = out0[0:64, 8].astype(np.int64).reshape(2, 32)
    gjs = out0[0:64, 9].astype(np.int64).reshape(2, 32)
    tok = np.asarray(gt_tokens).astype(np.int64)

    bidx = np.arange(B)[:, None, None]
    lidx = np.arange(LM1)[None, None, :]
    tgt = tok[bidx, gjs[:, :, None], lidx + 1]  # (B, S, LM1)
    tlog = caption_logits[bidx, pis[:, :, None], lidx, tgt].astype(np.float64)
    ce = (lse_bsl - tlog).mean(axis=2)  # (B, S)
    cap = np.clip(np.clip(ce, 0.0, None).mean(axis=1), 0.0, None)  # (B,)
    bbox = out0[[0, 32], 10].astype(np.float64)
    obj = out0[[0, 32], 11].astype(np.float64)
    total = max((5.0 * bbox + 0.1 * cap + obj).mean(), 0.0)
    comps = [5.0 * bbox.mean(), 0.1 * cap.mean(), obj.mean()]
    return np.array([total] + comps, np.float32)


# ---------------- entry points ----------------

V8_FULL = 4000
NC_CORES = 8
_CACHE = {}


def get_nc(V8=V8_FULL):
    key = V8
    if key not in _CACHE:
        _CACHE[key] = build_nc(V8, num_devices=NC_CORES)
    return _CACHE[key]


def run_device(in_maps, V8=V8_FULL, trace=False, **kw):
    from concourse.bass_utils import run_bass_kernel_spmd

    nc = get_nc(V8)
    return run_bass_kernel_spmd(
        nc, in_maps, core_ids=list(range(NC_CORES)), trace=trace, **kw)


def kernel(pred_boxes, pred_objectness, caption_logits, gt_boxes, gt_tokens):
    pred_boxes = np.asarray(pred_boxes, np.float32)
    pred_objectness = np.asarray(pred_objectness, np.float32)
    caption_logits = np.asarray(caption_logits, np.float32)
    gt_boxes = np.asarray(gt_boxes, np.float32)
    in_maps = shard_inputs(
        pred_boxes, pred_objectness, caption_logits, gt_boxes, V8_FULL, NC_CORES)
    res = run_device(in_maps)
    outs = [r["out"] for r in res.results]
    return combine(outs, caption_logits, gt_tokens, V8_FULL, NC_CORES)
